# revision 19
# baseline (speedup 1.0000x reference)
"""Trainium2 Bass kernel for one attention-LSTM decoder step.

dims: B=512, S=64, H=1024, E=256, V=128, L=2, sharded data-parallel over
batch across 8 NeuronCores (64 batches/core). All matmuls run in fp16 with
fp32 PSUM accumulation; elementwise/softmax math in fp32.
"""

import sys

if "/opt/trn_rl_repo" not in sys.path:
    sys.path.insert(0, "/opt/trn_rl_repo")

import numpy as np

import concourse.bacc as bacc
import concourse.mybir as mybir
import concourse.tile as tile
from concourse.bass_utils import run_bass_kernel_spmd

B, S, H, E, V = 512, 64, 1024, 256, 128
NCORES = 8
BC = B // NCORES          # 64 batches per core
TOK = BC * S              # 4096 tokens per core
F = E + H                 # 1280 LSTM input features
G4 = 4 * H                # 4096 gate rows
F16 = mybir.dt.float16
F32 = mybir.dt.float32
AF = mybir.ActivationFunctionType
ALU = mybir.AluOpType
AX = mybir.AxisListType

_COMPILED = [None]


def _build():
    nc = bacc.Bacc("TRN2", target_bir_lowering=False, debug=False,
                   num_devices=NCORES)

    # ---- DRAM I/O ----
    d_encT = nc.dram_tensor("encT", [H, TOK], F16, kind="ExternalInput")
    d_encS = nc.dram_tensor("encS", [TOK, H], F16, kind="ExternalInput")
    d_U = nc.dram_tensor("Umat", [H, H], F16, kind="ExternalInput")
    d_W = nc.dram_tensor("Wmat", [H, H], F16, kind="ExternalInput")
    d_Vw = nc.dram_tensor("VwR", [128, 8], F16, kind="ExternalInput")
    d_hT0 = nc.dram_tensor("hT0", [128, 8 * BC], F16, kind="ExternalInput")
    d_hT1 = nc.dram_tensor("hT1", [128, 8 * BC], F16, kind="ExternalInput")
    d_ohT = nc.dram_tensor("onehotT", [V, BC], F16, kind="ExternalInput")
    d_emb = nc.dram_tensor("embW", [V, E], F16, kind="ExternalInput")
    d_oh64 = nc.dram_tensor("oh64", [BC, TOK], F16, kind="ExternalInput")
    d_I2 = nc.dram_tensor("I2", [128, 64], F32, kind="ExternalInput")
    d_I128 = nc.dram_tensor("I128", [128, 128], F16, kind="ExternalInput")
    d_ones1 = nc.dram_tensor("ones1", [1, BC], F16, kind="ExternalInput")
    d_wi0 = nc.dram_tensor("Wih0T", [F, G4], F16, kind="ExternalInput")
    d_wh0 = nc.dram_tensor("Whh0T", [H, G4], F16, kind="ExternalInput")
    d_wi1 = nc.dram_tensor("Wih1T", [H, G4], F16, kind="ExternalInput")
    d_wh1 = nc.dram_tensor("Whh1T", [H, G4], F16, kind="ExternalInput")
    d_b0 = nc.dram_tensor("bias0", [1, G4], F16, kind="ExternalInput")
    d_b1 = nc.dram_tensor("bias1", [1, G4], F16, kind="ExternalInput")
    d_c0 = nc.dram_tensor("cell0", [BC, H], F32, kind="ExternalInput")
    d_c1 = nc.dram_tensor("cell1", [BC, H], F32, kind="ExternalInput")
    d_owT = nc.dram_tensor("outWT", [H, V], F16, kind="ExternalInput")
    d_ob = nc.dram_tensor("outB", [1, V], F16, kind="ExternalInput")

    d_logits = nc.dram_tensor("logits", [BC, V], F32, kind="ExternalOutput")
    d_h0n = nc.dram_tensor("h0n", [BC, H], F32, kind="ExternalOutput")
    d_h1n = nc.dram_tensor("h1n", [BC, H], F32, kind="ExternalOutput")
    d_c0n = nc.dram_tensor("c0n", [BC, H], F32, kind="ExternalOutput")
    d_c1n = nc.dram_tensor("c1n", [BC, H], F32, kind="ExternalOutput")

    with tile.TileContext(nc) as tc:
        with (
            tc.tile_pool(name="const", bufs=1) as cpool,
            tc.tile_pool(name="stream", bufs=3) as stp,
            tc.tile_pool(name="tanh", bufs=2) as thp,
            tc.tile_pool(name="wls", bufs=3) as wpool,
            tc.tile_pool(name="mid", bufs=1) as mid,
            tc.tile_pool(name="psA", bufs=2, space="PSUM") as psA,
            tc.tile_pool(name="psE", bufs=2, space="PSUM") as psE,
            tc.tile_pool(name="psS", bufs=2, space="PSUM") as psS,
            tc.tile_pool(name="psT", bufs=2, space="PSUM") as psT,
        ):
            # ---------- resident constants ----------
            vw_sb = cpool.tile([128, 8], F16, tag="vw")
            nc.sync.dma_start(vw_sb[:], d_Vw[:])
            hT0_sb = cpool.tile([128, 8 * BC], F16, tag="hT0")
            nc.sync.dma_start(hT0_sb[:], d_hT0[:])
            hT1_sb = cpool.tile([128, 8 * BC], F16, tag="hT1")
            nc.sync.dma_start(hT1_sb[:], d_hT1[:])
            ohT_sb = cpool.tile([V, BC], F16, tag="ohT")
            nc.sync.dma_start(ohT_sb[:], d_ohT[:])
            emb_sb = cpool.tile([V, E], F16, tag="emb")
            nc.sync.dma_start(emb_sb[:], d_emb[:])
            ones1_sb = cpool.tile([1, BC], F16, tag="ones1")
            nc.sync.dma_start(ones1_sb[:], d_ones1[:])

            # ---------- phase A: t2 = h_top @ W  -> [BC, H] f16 ----------
            t2_sb = mid.tile([BC, H], F16, tag="t2")
            for n2 in range(2):
                ps = psS.tile([BC, 512], F32, tag="ps64", name="t2ps")
                wsl = stp.tile([128, 4096], F16, tag="big", name="wsl")
                nc.sync.dma_start(
                    wsl[:].rearrange("p (k c) -> p k c", c=512),
                    d_W.rearrange("(k p) h -> k p h", p=128)
                       [:, :, 512 * n2:512 * (n2 + 1)]
                       .transpose([1, 0, 2]))
                for k in range(8):
                    nc.tensor.matmul(
                        ps[:], hT1_sb[:, 64 * k:64 * (k + 1)],
                        wsl[:, 512 * k:512 * (k + 1)],
                        start=(k == 0), stop=(k == 7))
                nc.vector.tensor_copy(t2_sb[:, 512 * n2:512 * (n2 + 1)], ps[:])

            U_sb = [cpool.tile([128, H], F16, tag=f"U{k}", name=f"U{k}") for k in range(8)]
            for k in range(8):
                nc.scalar.dma_start(U_sb[k][:], d_U[128 * k:128 * (k + 1), :])
            # ---------- phase B: embedded^T -> xT blocks 0..1 ----------
            xT_sb = mid.tile([128, 10 * BC], F16, tag="xT")
            for et in range(2):
                ps = psT.tile([128, BC], F32, tag="pT", name="embps")
                nc.tensor.matmul(ps[:], emb_sb[:, 128 * et:128 * (et + 1)],
                                 ohT_sb[:], start=True, stop=True)
                nc.vector.tensor_copy(xT_sb[:, BC * et:BC * (et + 1)], ps[:])

            def load_et(n):
                t = stp.tile([128, 4096], F16, tag="big", name=f"et{n}")
                nc.sync.dma_start(
                    t[:].rearrange("p (k c) -> p k c", c=512),
                    d_encT.rearrange("(k p) t -> k p t", p=128)
                          [:, :, 512 * n:512 * (n + 1)]
                          .transpose([1, 0, 2]))
                return t

            def load_oh(n):
                t = stp.tile([BC, 512], F16, tag="oh", name=f"oh{n}")
                nc.sync.dma_start(t[:], d_oh64[:, 512 * n:512 * (n + 1)])
                return t

            pre_et = {n: load_et(n) for n in range(2)}
            pre_oh = {n: load_oh(n) for n in range(2)}
            def load_w(wd, n_k, n, eng):
                t = wpool.tile([128, n_k * 512], F16,
                               tag="wx" if n_k > 8 else "wh", name=f"w{n}")
                eng.dma_start(
                    t[:].rearrange("p (k c) -> p k c", c=512),
                    wd.rearrange("(k p) g -> k p g", p=128)
                      [:, :, 512 * n:512 * (n + 1)].transpose([1, 0, 2]))
                return t

            pre_wx0 = {n: load_w(d_wi0, 10, n, nc.scalar) for n in range(3)}
            pre_wh0 = {n: load_w(d_wh0, 8, n, nc.sync) for n in range(3)}

            # ---------- phase C: t1 + t2 -> tanh -> scores -> softmax -> ct ---
            # token tile n holds ALL 64 s-positions of batches 8n..8n+8, so
            # each tile's softmax + attention-context can be computed inline
            # against the already-resident encT tile (no second enc stream).
            ctT_sb = mid.tile([128, 512], F32, tag="ctT")  # [h-blk k][8n+b] cols
            for n in range(8):          # token tiles (512 tokens, 8 batches)
                et = pre_et.pop(n) if n in pre_et else load_et(n)
                oh64_t = pre_oh.pop(n) if n in pre_oh else load_oh(n)
                pe = psE.tile([1, 512], F32, tag="eps")
                for m in range(8):      # output-H tiles
                    pt = psA.tile([128, 512], F32, tag="t1ps")
                    for k in range(8):
                        nc.tensor.matmul(
                            pt[:], U_sb[k][:, 128 * m:128 * (m + 1)],
                            et[:, 512 * k:512 * (k + 1)],
                            start=(k == 0), stop=False)
                    # inject t2 broadcast over s:  lhsT=[64b,128h'] rhs=[64b,512tok]
                    nc.tensor.matmul(
                        pt[:], t2_sb[:, 128 * m:128 * (m + 1)],
                        oh64_t[:], start=False, stop=True)
                    th = thp.tile([128, 512], F16, tag="tanh")
                    nc.scalar.activation(th[:], pt[:], AF.Tanh)
                    nc.tensor.matmul(pe[:], vw_sb[:, m:m + 1], th[:],
                                     start=(m == 0), stop=(m == 7))
                # --- inline softmax over s for batches 8n..8n+8 ---
                er = mid.tile([1, 512], F32, tag="er", name="er", bufs=2)
                nc.vector.tensor_copy(er[:], pe[:])
                eb = mid.tile([8, S], F32, tag="eb", name="eb", bufs=2)
                nc.sync.dma_start(
                    eb[:], er[0:1, :].rearrange("p (b s) -> p b s", b=8))
                mx = mid.tile([8, 1], F32, tag="mx", name="mx", bufs=2)
                nc.vector.tensor_reduce(mx[:], eb[:], axis=AX.X, op=ALU.max)
                negmx = mid.tile([8, 1], F32, tag="negmx", name="negmx", bufs=2)
                nc.vector.tensor_scalar_mul(negmx[:], mx[:], -1.0)
                pb = mid.tile([8, S], F32, tag="pb", name="pb", bufs=2)
                nc.scalar.activation(pb[:], eb[:], AF.Exp, bias=negmx[:])
                sm = mid.tile([8, 1], F32, tag="sm", name="sm", bufs=2)
                nc.vector.tensor_reduce(sm[:], pb[:], axis=AX.X, op=ALU.add)
                rinv = mid.tile([8, 1], F32, tag="rinv", name="rinv", bufs=2)
                nc.vector.reciprocal(rinv[:], sm[:])
                ab = mid.tile([8, S], F16, tag="ab", name="ab", bufs=2)
                nc.vector.tensor_scalar_mul(ab[:], pb[:], rinv[:])
                ar = mid.tile([1, 512], F16, tag="ar", name="ar", bufs=2)
                nc.sync.dma_start(
                    ar[0:1, :].rearrange("p (b s) -> p b s", b=8), ab[:])
                af = mid.tile([128, 512], F16, tag="af", name="af", bufs=2)
                nc.gpsimd.partition_broadcast(af[:], ar[:])
                # --- ct^T columns for these 8 batches, per h-block k ---
                for k in range(8):
                    eng = nc.vector if k < 5 else nc.gpsimd
                    pr = thp.tile([128, 512], F16, tag="prod", name="pr", bufs=3)
                    eng.tensor_tensor(
                        pr[:], et[:, 512 * k:512 * (k + 1)], af[:], ALU.mult)
                    nc.vector.tensor_reduce(
                        ctT_sb[:, 64 * k + 8 * n:64 * k + 8 * n + 8],
                        pr[:].rearrange("p (b s) -> p b s", b=8),
                        axis=AX.X, op=ALU.add)
            # ct^T blocked [h-blk][b] is exactly xT blocks 2..9
            nc.vector.tensor_copy(xT_sb[:, 2 * BC:10 * BC], ctT_sb[:])

            I128_sb = cpool.tile([128, 128], F16, tag="I128")
            nc.sync.dma_start(I128_sb[:], d_I128[:])
            c0_sb = cpool.tile([BC, H], F32, tag="c0")
            nc.sync.dma_start(c0_sb[:], d_c0[:])
            c1_sb = cpool.tile([BC, H], F32, tag="c1")
            nc.sync.dma_start(c1_sb[:], d_c1[:])
            owT_sb = [cpool.tile([128, V], F16, tag=f"ow{k}", name=f"ow{k}") for k in range(8)]
            for k in range(8):
                nc.sync.dma_start(owT_sb[k][:], d_owT[128 * k:128 * (k + 1), :])
            ob_sb = cpool.tile([1, V], F16, tag="ob")
            nc.sync.dma_start(ob_sb[:], d_ob[:])

            # ---------- phases G/H: two LSTM layers ----------
            def lstm_layer(xT, n_xk, wxd, whd, hT, bias_d, c_in,
                           d_hout, d_cout, hTout, lname, pre_wx, pre_wh):
                """xT: SBUF [128, n_xk*64] input^T blocks; whd/wxd DRAM weights;
                hT: SBUF [128, 8*64] prev-h^T blocks; returns nothing."""
                gates = mid.tile([BC, G4], F16, tag="gates", name="gates")
                bt = mid.tile([1, G4], F16, tag="btile", name="bt")
                nc.sync.dma_start(bt[:], bias_d[:])
                for n in range(8):
                    wx = pre_wx.pop(n) if n in pre_wx else \
                        load_w(wxd, n_xk, n, nc.scalar)
                    wh = pre_wh.pop(n) if n in pre_wh else \
                        load_w(whd, 8, n, nc.sync)
                    ps = psS.tile([BC, 512], F32, tag="ps64", name="gps")
                    for k in range(8):
                        nc.tensor.matmul(ps[:], hT[:, 64 * k:64 * (k + 1)],
                                         wh[:, 512 * k:512 * (k + 1)],
                                         start=(k == 0), stop=False)
                    nc.tensor.matmul(ps[:], ones1_sb[:],
                                     bt[:, 512 * n:512 * (n + 1)],
                                     start=False, stop=False)
                    for k in range(n_xk):
                        nc.tensor.matmul(ps[:], xT[:, 64 * k:64 * (k + 1)],
                                         wx[:, 512 * k:512 * (k + 1)],
                                         start=False, stop=(k == n_xk - 1))
                    func = AF.Tanh if n in (4, 5) else AF.Sigmoid
                    nc.scalar.activation(gates[:, 512 * n:512 * (n + 1)],
                                         ps[:], func)
                # c2 = sig_f*c + sig_i*tanh_g ; h2 = sig_o*tanh(c2)
                tmp = mid.tile([BC, H], F32, tag="lstm_tmp", name="tmp")
                nc.vector.tensor_tensor(tmp[:], gates[:, 0:H],
                                        gates[:, 2 * H:3 * H], ALU.mult)
                c2 = mid.tile([BC, H], F32, tag="c2t", name="c2")
                nc.vector.tensor_tensor(c2[:], gates[:, H:2 * H], c_in[:],
                                        ALU.mult)
                nc.vector.tensor_tensor(c2[:], c2[:], tmp[:], ALU.add)
                nc.sync.dma_start(d_cout[:], c2[:])
                tc2 = mid.tile([BC, H], F32, tag="lstm_tmp", name="tc2")
                nc.scalar.activation(tc2[:], c2[:], AF.Tanh)
                h2 = mid.tile([BC, H], F32, tag="h2t", name="h2")
                nc.vector.tensor_tensor(h2[:], gates[:, 3 * H:4 * H], tc2[:],
                                        ALU.mult)
                nc.sync.dma_start(d_hout[:], h2[:])
                h2f = mid.tile([BC, H], F16, tag="lstm_h2f", name="h2f")
                nc.vector.tensor_copy(h2f[:], h2[:])
                for j in range(8):
                    pt = psT.tile([128, BC], F16, tag="pT", name="trps")
                    nc.tensor.transpose(pt[:], h2f[:, 128 * j:128 * (j + 1)],
                                        I128_sb[0:64, 0:64])
                    nc.vector.tensor_copy(hTout[:, BC * j:BC * (j + 1)], pt[:])

            h0T_sb = mid.tile([128, 8 * BC], F16, tag="h0T")
            lstm_layer(xT_sb, 10, d_wi0, d_wh0, hT0_sb, d_b0, c0_sb,
                       d_h0n, d_c0n, h0T_sb, "l0", pre_wx0, pre_wh0)
            h1T_sb = mid.tile([128, 8 * BC], F16, tag="h1T")
            lstm_layer(h0T_sb, 8, d_wi1, d_wh1, hT1_sb, d_b1, c1_sb,
                       d_h1n, d_c1n, h1T_sb, "l1", {}, {})

            # ---------- phase I: logits ----------
            pl = psS.tile([BC, V], F32, tag="ps64", name="lps")
            for k in range(8):
                nc.tensor.matmul(pl[:], h1T_sb[:, 64 * k:64 * (k + 1)],
                                 owT_sb[k][:], start=(k == 0), stop=False)
            nc.tensor.matmul(pl[:], ones1_sb[:], ob_sb[:],
                             start=False, stop=True)
            lo = mid.tile([BC, V], F32, tag="lo")
            nc.vector.tensor_copy(lo[:], pl[:])
            nc.sync.dma_start(d_logits[:], lo[:])

    nc.compile()
    return nc


def _prep_inputs(input_ids, hidden, cell, encoder_outputs, emb, U, W, Vw,
                 Wih0, Whh0, bih0, bhh0, Wih1, Whh1, bih1, bhh1,
                 out_w, out_b):
    f16 = np.float16
    # shared across cores
    U16 = np.ascontiguousarray(U.astype(f16))
    W16 = np.ascontiguousarray(W.astype(f16))
    VwR = np.ascontiguousarray(Vw.reshape(8, 128).T.astype(f16))  # [128,8]
    emb16 = np.ascontiguousarray(emb.astype(f16))
    oh64 = np.zeros((BC, TOK), f16)
    for b in range(BC):
        oh64[b, 64 * b:64 * (b + 1)] = 1.0
    I2 = np.zeros((128, 64), np.float32)
    I2[np.arange(128), np.arange(128) % 64] = 1.0
    I128 = np.eye(128, dtype=f16)
    ones1 = np.ones((1, BC), f16)
    Wih0T = np.ascontiguousarray(Wih0.T.astype(f16))
    Whh0T = np.ascontiguousarray(Whh0.T.astype(f16))
    Wih1T = np.ascontiguousarray(Wih1.T.astype(f16))
    Whh1T = np.ascontiguousarray(Whh1.T.astype(f16))
    b0 = np.ascontiguousarray((bih0 + bhh0)[None, :].astype(f16))
    b1 = np.ascontiguousarray((bih1 + bhh1)[None, :].astype(f16))
    owT = np.ascontiguousarray(out_w.T.astype(f16))
    ob = np.ascontiguousarray(out_b[None, :].astype(f16))

    def blocked_T(x):  # [BC,H] -> [128, 8*BC] (k-blocks of columns)
        t = np.ascontiguousarray(x.T)          # [H, BC]
        return np.ascontiguousarray(
            t.reshape(8, 128, BC).transpose(1, 0, 2).reshape(128, 8 * BC)
        ).astype(f16)

    ids = np.asarray(input_ids).reshape(B)
    in_maps = []
    for c in range(NCORES):
        bs = slice(BC * c, BC * (c + 1))
        enc_c = encoder_outputs[bs]                      # [BC, S, H]
        encT = np.ascontiguousarray(
            enc_c.reshape(TOK, H).T.astype(f16))         # [H, TOK] b-major
        encS = np.ascontiguousarray(
            enc_c.transpose(1, 0, 2).reshape(TOK, H).astype(f16))  # s-major
        ohT = np.zeros((V, BC), f16)
        ohT[ids[bs].astype(np.int64), np.arange(BC)] = 1.0
        in_maps.append({
            "encT": encT, "encS": encS, "Umat": U16, "Wmat": W16,
            "VwR": VwR,
            "hT0": blocked_T(hidden[0][bs]),
            "hT1": blocked_T(hidden[1][bs]),
            "onehotT": ohT, "embW": emb16, "oh64": oh64, "I2": I2,
            "I128": I128, "ones1": ones1,
            "Wih0T": Wih0T, "Whh0T": Whh0T, "Wih1T": Wih1T, "Whh1T": Whh1T,
            "bias0": b0, "bias1": b1,
            "cell0": np.ascontiguousarray(cell[0][bs], dtype=np.float32),
            "cell1": np.ascontiguousarray(cell[1][bs], dtype=np.float32),
            "outWT": owT, "outB": ob,
        })
    return in_maps


def kernel(input_ids, hidden, cell, encoder_outputs, emb, U, W, Vw,
           Wih0, Whh0, bih0, bhh0, Wih1, Whh1, bih1, bhh1,
           out_w, out_b, matrix=0, _trace=False):
    if _COMPILED[0] is None:
        _COMPILED[0] = _build()
    nc = _COMPILED[0]
    args = [np.asarray(a) for a in
            (input_ids, hidden, cell, encoder_outputs, emb, U, W, Vw,
             Wih0, Whh0, bih0, bhh0, Wih1, Whh1, bih1, bhh1, out_w, out_b)]
    in_maps = _prep_inputs(*args)
    res = run_bass_kernel_spmd(nc, in_maps, core_ids=list(range(NCORES)),
                               trace=_trace)
    outs = res.results
    logits = np.concatenate([outs[c]["logits"] for c in range(NCORES)], 0)
    h_new = np.stack([
        np.concatenate([outs[c]["h0n"] for c in range(NCORES)], 0),
        np.concatenate([outs[c]["h1n"] for c in range(NCORES)], 0)])
    c_new = np.stack([
        np.concatenate([outs[c]["c0n"] for c in range(NCORES)], 0),
        np.concatenate([outs[c]["c1n"] for c in range(NCORES)], 0)])
    out = logits[:, None, :].astype(np.float32)
    kernel._last_results = res
    if int(np.asarray(matrix)):
        raise NotImplementedError("matrix=1 path not needed (reference uses 0)")
    return (out, h_new.astype(np.float32), c_new.astype(np.float32))


# revision 20
# speedup vs baseline: 1.0888x; 1.0888x over previous
"""Trainium2 Bass kernel for one attention-LSTM decoder step.

dims: B=512, S=64, H=1024, E=256, V=128, L=2, sharded data-parallel over
batch across 8 NeuronCores (64 batches/core). All matmuls run in fp16 with
fp32 PSUM accumulation; elementwise/softmax math in fp32.
"""

import sys

if "/opt/trn_rl_repo" not in sys.path:
    sys.path.insert(0, "/opt/trn_rl_repo")

import numpy as np

import concourse.bacc as bacc
import concourse.mybir as mybir
import concourse.tile as tile
from concourse.bass_utils import run_bass_kernel_spmd

B, S, H, E, V = 512, 64, 1024, 256, 128
NCORES = 8
BC = B // NCORES          # 64 batches per core
TOK = BC * S              # 4096 tokens per core
F = E + H                 # 1280 LSTM input features
G4 = 4 * H                # 4096 gate rows
F16 = mybir.dt.float16
F32 = mybir.dt.float32
AF = mybir.ActivationFunctionType
ALU = mybir.AluOpType
AX = mybir.AxisListType

_COMPILED = [None]


def _build():
    nc = bacc.Bacc("TRN2", target_bir_lowering=False, debug=False,
                   num_devices=NCORES)

    # ---- DRAM I/O ----
    d_encT = nc.dram_tensor("encT", [H, TOK], F16, kind="ExternalInput")
    d_encS = nc.dram_tensor("encS", [TOK, H], F16, kind="ExternalInput")
    d_U = nc.dram_tensor("Umat", [H, H], F16, kind="ExternalInput")
    d_W = nc.dram_tensor("Wmat", [H, H], F16, kind="ExternalInput")
    d_Vw = nc.dram_tensor("VwR", [128, 8], F16, kind="ExternalInput")
    d_hT0 = nc.dram_tensor("hT0", [128, 8 * BC], F16, kind="ExternalInput")
    d_hT1 = nc.dram_tensor("hT1", [128, 8 * BC], F16, kind="ExternalInput")
    d_ohT = nc.dram_tensor("onehotT", [V, BC], F16, kind="ExternalInput")
    d_emb = nc.dram_tensor("embW", [V, E], F16, kind="ExternalInput")
    d_oh64 = nc.dram_tensor("oh64", [BC, TOK], F16, kind="ExternalInput")
    d_I2 = nc.dram_tensor("I2", [128, 64], F32, kind="ExternalInput")
    d_I128 = nc.dram_tensor("I128", [128, 128], F16, kind="ExternalInput")
    d_ones1 = nc.dram_tensor("ones1", [1, BC], F16, kind="ExternalInput")
    d_wi0 = nc.dram_tensor("Wih0T", [F, G4], F16, kind="ExternalInput")
    d_wh0 = nc.dram_tensor("Whh0T", [H, G4], F16, kind="ExternalInput")
    d_wi1 = nc.dram_tensor("Wih1T", [H, G4], F16, kind="ExternalInput")
    d_wh1 = nc.dram_tensor("Whh1T", [H, G4], F16, kind="ExternalInput")
    d_b0 = nc.dram_tensor("bias0", [1, G4], F16, kind="ExternalInput")
    d_b1 = nc.dram_tensor("bias1", [1, G4], F16, kind="ExternalInput")
    d_c0 = nc.dram_tensor("cell0", [BC, H], F32, kind="ExternalInput")
    d_c1 = nc.dram_tensor("cell1", [BC, H], F32, kind="ExternalInput")
    d_owT = nc.dram_tensor("outWT", [H, V], F16, kind="ExternalInput")
    d_ob = nc.dram_tensor("outB", [1, V], F16, kind="ExternalInput")

    d_logits = nc.dram_tensor("logits", [BC, V], F32, kind="ExternalOutput")
    d_h0n = nc.dram_tensor("h0n", [BC, H], F32, kind="ExternalOutput")
    d_h1n = nc.dram_tensor("h1n", [BC, H], F32, kind="ExternalOutput")
    d_c0n = nc.dram_tensor("c0n", [BC, H], F32, kind="ExternalOutput")
    d_c1n = nc.dram_tensor("c1n", [BC, H], F32, kind="ExternalOutput")

    with tile.TileContext(nc) as tc:
        with (
            tc.tile_pool(name="const", bufs=1) as cpool,
            tc.tile_pool(name="stream", bufs=3) as stp,
            tc.tile_pool(name="tanh", bufs=2) as thp,
            tc.tile_pool(name="wls", bufs=3) as wpool,
            tc.tile_pool(name="mid", bufs=1) as mid,
            tc.tile_pool(name="psA", bufs=2, space="PSUM") as psA,
            tc.tile_pool(name="psE", bufs=2, space="PSUM") as psE,
            tc.tile_pool(name="psS", bufs=2, space="PSUM") as psS,
            tc.tile_pool(name="psT", bufs=2, space="PSUM") as psT,
        ):
            # ---------- resident constants ----------
            vw_sb = cpool.tile([128, 8], F16, tag="vw")
            nc.sync.dma_start(vw_sb[:], d_Vw[:])
            hT0_sb = cpool.tile([128, 8 * BC], F16, tag="hT0")
            nc.sync.dma_start(hT0_sb[:], d_hT0[:])
            hT1_sb = cpool.tile([128, 8 * BC], F16, tag="hT1")
            nc.sync.dma_start(hT1_sb[:], d_hT1[:])
            ohT_sb = cpool.tile([V, BC], F16, tag="ohT")
            nc.sync.dma_start(ohT_sb[:], d_ohT[:])
            emb_sb = cpool.tile([V, E], F16, tag="emb")
            nc.sync.dma_start(emb_sb[:], d_emb[:])
            ones1_sb = cpool.tile([1, BC], F16, tag="ones1")
            nc.sync.dma_start(ones1_sb[:], d_ones1[:])

            # ---------- phase A: t2 = h_top @ W  -> [BC, H] f16 ----------
            t2_sb = mid.tile([BC, H], F16, tag="t2")
            for n2 in range(2):
                ps = psS.tile([BC, 512], F32, tag="ps64", name="t2ps")
                wsl = stp.tile([128, 4096], F16, tag="big", name="wsl")
                nc.sync.dma_start(
                    wsl[:].rearrange("p (k c) -> p k c", c=512),
                    d_W.rearrange("(k p) h -> k p h", p=128)
                       [:, :, 512 * n2:512 * (n2 + 1)]
                       .transpose([1, 0, 2]))
                for k in range(8):
                    nc.tensor.matmul(
                        ps[:], hT1_sb[:, 64 * k:64 * (k + 1)],
                        wsl[:, 512 * k:512 * (k + 1)],
                        start=(k == 0), stop=(k == 7))
                nc.vector.tensor_copy(t2_sb[:, 512 * n2:512 * (n2 + 1)], ps[:])

            U_sb = [cpool.tile([128, H], F16, tag=f"U{k}", name=f"U{k}") for k in range(8)]
            for k in range(8):
                nc.scalar.dma_start(U_sb[k][:], d_U[128 * k:128 * (k + 1), :])
            # ---------- phase B: embedded^T -> xT blocks 0..1 ----------
            xT_sb = mid.tile([128, 10 * BC], F16, tag="xT")
            for et in range(2):
                ps = psT.tile([128, BC], F32, tag="pT", name="embps")
                nc.tensor.matmul(ps[:], emb_sb[:, 128 * et:128 * (et + 1)],
                                 ohT_sb[:], start=True, stop=True)
                nc.vector.tensor_copy(xT_sb[:, BC * et:BC * (et + 1)], ps[:])

            def load_et(n):
                t = stp.tile([128, 4096], F16, tag="big", name=f"et{n}")
                nc.sync.dma_start(
                    t[:].rearrange("p (k c) -> p k c", c=512),
                    d_encT.rearrange("(k p) t -> k p t", p=128)
                          [:, :, 512 * n:512 * (n + 1)]
                          .transpose([1, 0, 2]))
                return t

            def load_oh(n):
                t = stp.tile([BC, 512], F16, tag="oh", name=f"oh{n}")
                nc.sync.dma_start(t[:], d_oh64[:, 512 * n:512 * (n + 1)])
                return t

            pre_et = {n: load_et(n) for n in range(2)}
            pre_oh = {n: load_oh(n) for n in range(2)}
            def load_w(wd, n_k, n, eng):
                t = wpool.tile([128, n_k * 512], F16,
                               tag="wx" if n_k > 8 else "wh", name=f"w{n}")
                eng.dma_start(
                    t[:].rearrange("p (k c) -> p k c", c=512),
                    wd.rearrange("(k p) g -> k p g", p=128)
                      [:, :, 512 * n:512 * (n + 1)].transpose([1, 0, 2]))
                return t

            pre_wx0 = {n: load_w(d_wi0, 10, n, nc.scalar) for n in range(3)}
            pre_wh0 = {n: load_w(d_wh0, 8, n, nc.sync) for n in range(3)}

            # ---------- phase C: t1 + t2 -> tanh -> scores -> softmax -> ct ---
            # token tile n holds ALL 64 s-positions of batches 8n..8n+8, so
            # each tile's softmax + attention-context can be computed inline
            # against the already-resident encT tile (no second enc stream).
            ctT_sb = mid.tile([128, 512], F32, tag="ctT")  # [h-blk k][8n+b] cols
            for n in range(8):          # token tiles (512 tokens, 8 batches)
                et = pre_et.pop(n) if n in pre_et else load_et(n)
                oh64_t = pre_oh.pop(n) if n in pre_oh else load_oh(n)
                pe = psE.tile([1, 512], F32, tag="eps")
                for m in range(8):      # output-H tiles
                    pt = psA.tile([128, 512], F32, tag="t1ps")
                    for k in range(8):
                        nc.tensor.matmul(
                            pt[:], U_sb[k][:, 128 * m:128 * (m + 1)],
                            et[:, 512 * k:512 * (k + 1)],
                            start=(k == 0), stop=False)
                    # inject t2 broadcast over s:  lhsT=[64b,128h'] rhs=[64b,512tok]
                    nc.tensor.matmul(
                        pt[:], t2_sb[:, 128 * m:128 * (m + 1)],
                        oh64_t[:], start=False, stop=True)
                    th = thp.tile([128, 512], F16, tag="tanh")
                    nc.scalar.activation(th[:], pt[:], AF.Tanh)
                    nc.tensor.matmul(pe[:], vw_sb[:, m:m + 1], th[:],
                                     start=(m == 0), stop=(m == 7))
                # --- inline softmax over s for batches 8n..8n+8 ---
                er = mid.tile([1, 512], F32, tag="er", name="er", bufs=2)
                nc.vector.tensor_copy(er[:], pe[:])
                eb = mid.tile([8, S], F32, tag="eb", name="eb", bufs=2)
                nc.gpsimd.dma_start(
                    eb[:], er[0:1, :].rearrange("p (b s) -> p b s", b=8))
                mx = mid.tile([8, 1], F32, tag="mx", name="mx", bufs=2)
                nc.vector.tensor_reduce(mx[:], eb[:], axis=AX.X, op=ALU.max)
                negmx = mid.tile([8, 1], F32, tag="negmx", name="negmx", bufs=2)
                nc.vector.tensor_scalar_mul(negmx[:], mx[:], -1.0)
                pb = mid.tile([8, S], F32, tag="pb", name="pb", bufs=2)
                nc.scalar.activation(pb[:], eb[:], AF.Exp, bias=negmx[:])
                sm = mid.tile([8, 1], F32, tag="sm", name="sm", bufs=2)
                nc.vector.tensor_reduce(sm[:], pb[:], axis=AX.X, op=ALU.add)
                rinv = mid.tile([8, 1], F32, tag="rinv", name="rinv", bufs=2)
                nc.vector.reciprocal(rinv[:], sm[:])
                ab = mid.tile([8, S], F16, tag="ab", name="ab", bufs=2)
                nc.vector.tensor_scalar_mul(ab[:], pb[:], rinv[:])
                ar = mid.tile([1, 512], F16, tag="ar", name="ar", bufs=2)
                nc.gpsimd.dma_start(
                    ar[0:1, :].rearrange("p (b s) -> p b s", b=8), ab[:])
                af = mid.tile([128, 512], F16, tag="af", name="af", bufs=2)
                nc.gpsimd.partition_broadcast(af[:], ar[:])
                # --- ct^T columns for these 8 batches, per h-block k ---
                for k in range(8):
                    pr = thp.tile([128, 512], F16, tag="prod", name="pr", bufs=3)
                    nc.vector.tensor_tensor(
                        pr[:], et[:, 512 * k:512 * (k + 1)], af[:], ALU.mult)
                    nc.vector.tensor_reduce(
                        ctT_sb[:, 64 * k + 8 * n:64 * k + 8 * n + 8],
                        pr[:].rearrange("p (b s) -> p b s", b=8),
                        axis=AX.X, op=ALU.add)
            # ct^T blocked [h-blk][b] is exactly xT blocks 2..9
            nc.vector.tensor_copy(xT_sb[:, 2 * BC:10 * BC], ctT_sb[:])

            I128_sb = cpool.tile([128, 128], F16, tag="I128")
            nc.sync.dma_start(I128_sb[:], d_I128[:])
            c0_sb = cpool.tile([BC, H], F32, tag="c0")
            nc.sync.dma_start(c0_sb[:], d_c0[:])
            c1_sb = cpool.tile([BC, H], F32, tag="c1")
            nc.sync.dma_start(c1_sb[:], d_c1[:])
            owT_sb = [cpool.tile([128, V], F16, tag=f"ow{k}", name=f"ow{k}") for k in range(8)]
            for k in range(8):
                nc.sync.dma_start(owT_sb[k][:], d_owT[128 * k:128 * (k + 1), :])
            ob_sb = cpool.tile([1, V], F16, tag="ob")
            nc.sync.dma_start(ob_sb[:], d_ob[:])

            # ---------- phases G/H: two LSTM layers ----------
            def lstm_layer(xT, n_xk, wxd, whd, hT, bias_d, c_in,
                           d_hout, d_cout, hTout, lname, pre_wx, pre_wh):
                """xT: SBUF [128, n_xk*64] input^T blocks; whd/wxd DRAM weights;
                hT: SBUF [128, 8*64] prev-h^T blocks; returns nothing."""
                gates = mid.tile([BC, G4], F16, tag="gates", name="gates")
                bt = mid.tile([1, G4], F16, tag="btile", name="bt")
                nc.sync.dma_start(bt[:], bias_d[:])
                for n in range(8):
                    wx = pre_wx.pop(n) if n in pre_wx else \
                        load_w(wxd, n_xk, n, nc.scalar)
                    wh = pre_wh.pop(n) if n in pre_wh else \
                        load_w(whd, 8, n, nc.sync)
                    ps = psS.tile([BC, 512], F32, tag="ps64", name="gps")
                    for k in range(8):
                        nc.tensor.matmul(ps[:], hT[:, 64 * k:64 * (k + 1)],
                                         wh[:, 512 * k:512 * (k + 1)],
                                         start=(k == 0), stop=False)
                    nc.tensor.matmul(ps[:], ones1_sb[:],
                                     bt[:, 512 * n:512 * (n + 1)],
                                     start=False, stop=False)
                    for k in range(n_xk):
                        nc.tensor.matmul(ps[:], xT[:, 64 * k:64 * (k + 1)],
                                         wx[:, 512 * k:512 * (k + 1)],
                                         start=False, stop=(k == n_xk - 1))
                    func = AF.Tanh if n in (4, 5) else AF.Sigmoid
                    nc.scalar.activation(gates[:, 512 * n:512 * (n + 1)],
                                         ps[:], func)
                # c2 = sig_f*c + sig_i*tanh_g ; h2 = sig_o*tanh(c2)
                tmp = mid.tile([BC, H], F32, tag="lstm_tmp", name="tmp")
                nc.vector.tensor_tensor(tmp[:], gates[:, 0:H],
                                        gates[:, 2 * H:3 * H], ALU.mult)
                c2 = mid.tile([BC, H], F32, tag="c2t", name="c2")
                nc.vector.tensor_tensor(c2[:], gates[:, H:2 * H], c_in[:],
                                        ALU.mult)
                nc.vector.tensor_tensor(c2[:], c2[:], tmp[:], ALU.add)
                nc.sync.dma_start(d_cout[:], c2[:])
                tc2 = mid.tile([BC, H], F32, tag="lstm_tmp", name="tc2")
                nc.scalar.activation(tc2[:], c2[:], AF.Tanh)
                h2 = mid.tile([BC, H], F32, tag="h2t", name="h2")
                nc.vector.tensor_tensor(h2[:], gates[:, 3 * H:4 * H], tc2[:],
                                        ALU.mult)
                nc.sync.dma_start(d_hout[:], h2[:])
                h2f = mid.tile([BC, H], F16, tag="lstm_h2f", name="h2f")
                nc.vector.tensor_copy(h2f[:], h2[:])
                for j in range(8):
                    pt = psT.tile([128, BC], F16, tag="pT", name="trps")
                    nc.tensor.transpose(pt[:], h2f[:, 128 * j:128 * (j + 1)],
                                        I128_sb[0:64, 0:64])
                    nc.vector.tensor_copy(hTout[:, BC * j:BC * (j + 1)], pt[:])

            h0T_sb = mid.tile([128, 8 * BC], F16, tag="h0T")
            lstm_layer(xT_sb, 10, d_wi0, d_wh0, hT0_sb, d_b0, c0_sb,
                       d_h0n, d_c0n, h0T_sb, "l0", pre_wx0, pre_wh0)
            h1T_sb = mid.tile([128, 8 * BC], F16, tag="h1T")
            lstm_layer(h0T_sb, 8, d_wi1, d_wh1, hT1_sb, d_b1, c1_sb,
                       d_h1n, d_c1n, h1T_sb, "l1", {}, {})

            # ---------- phase I: logits ----------
            pl = psS.tile([BC, V], F32, tag="ps64", name="lps")
            for k in range(8):
                nc.tensor.matmul(pl[:], h1T_sb[:, 64 * k:64 * (k + 1)],
                                 owT_sb[k][:], start=(k == 0), stop=False)
            nc.tensor.matmul(pl[:], ones1_sb[:], ob_sb[:],
                             start=False, stop=True)
            lo = mid.tile([BC, V], F32, tag="lo")
            nc.vector.tensor_copy(lo[:], pl[:])
            nc.sync.dma_start(d_logits[:], lo[:])

    nc.compile()
    return nc


def _prep_inputs(input_ids, hidden, cell, encoder_outputs, emb, U, W, Vw,
                 Wih0, Whh0, bih0, bhh0, Wih1, Whh1, bih1, bhh1,
                 out_w, out_b):
    f16 = np.float16
    # shared across cores
    U16 = np.ascontiguousarray(U.astype(f16))
    W16 = np.ascontiguousarray(W.astype(f16))
    VwR = np.ascontiguousarray(Vw.reshape(8, 128).T.astype(f16))  # [128,8]
    emb16 = np.ascontiguousarray(emb.astype(f16))
    oh64 = np.zeros((BC, TOK), f16)
    for b in range(BC):
        oh64[b, 64 * b:64 * (b + 1)] = 1.0
    I2 = np.zeros((128, 64), np.float32)
    I2[np.arange(128), np.arange(128) % 64] = 1.0
    I128 = np.eye(128, dtype=f16)
    ones1 = np.ones((1, BC), f16)
    Wih0T = np.ascontiguousarray(Wih0.T.astype(f16))
    Whh0T = np.ascontiguousarray(Whh0.T.astype(f16))
    Wih1T = np.ascontiguousarray(Wih1.T.astype(f16))
    Whh1T = np.ascontiguousarray(Whh1.T.astype(f16))
    b0 = np.ascontiguousarray((bih0 + bhh0)[None, :].astype(f16))
    b1 = np.ascontiguousarray((bih1 + bhh1)[None, :].astype(f16))
    owT = np.ascontiguousarray(out_w.T.astype(f16))
    ob = np.ascontiguousarray(out_b[None, :].astype(f16))

    def blocked_T(x):  # [BC,H] -> [128, 8*BC] (k-blocks of columns)
        t = np.ascontiguousarray(x.T)          # [H, BC]
        return np.ascontiguousarray(
            t.reshape(8, 128, BC).transpose(1, 0, 2).reshape(128, 8 * BC)
        ).astype(f16)

    ids = np.asarray(input_ids).reshape(B)
    in_maps = []
    for c in range(NCORES):
        bs = slice(BC * c, BC * (c + 1))
        enc_c = encoder_outputs[bs]                      # [BC, S, H]
        encT = np.ascontiguousarray(
            enc_c.reshape(TOK, H).T.astype(f16))         # [H, TOK] b-major
        encS = np.ascontiguousarray(
            enc_c.transpose(1, 0, 2).reshape(TOK, H).astype(f16))  # s-major
        ohT = np.zeros((V, BC), f16)
        ohT[ids[bs].astype(np.int64), np.arange(BC)] = 1.0
        in_maps.append({
            "encT": encT, "encS": encS, "Umat": U16, "Wmat": W16,
            "VwR": VwR,
            "hT0": blocked_T(hidden[0][bs]),
            "hT1": blocked_T(hidden[1][bs]),
            "onehotT": ohT, "embW": emb16, "oh64": oh64, "I2": I2,
            "I128": I128, "ones1": ones1,
            "Wih0T": Wih0T, "Whh0T": Whh0T, "Wih1T": Wih1T, "Whh1T": Whh1T,
            "bias0": b0, "bias1": b1,
            "cell0": np.ascontiguousarray(cell[0][bs], dtype=np.float32),
            "cell1": np.ascontiguousarray(cell[1][bs], dtype=np.float32),
            "outWT": owT, "outB": ob,
        })
    return in_maps


def kernel(input_ids, hidden, cell, encoder_outputs, emb, U, W, Vw,
           Wih0, Whh0, bih0, bhh0, Wih1, Whh1, bih1, bhh1,
           out_w, out_b, matrix=0, _trace=False):
    if _COMPILED[0] is None:
        _COMPILED[0] = _build()
    nc = _COMPILED[0]
    args = [np.asarray(a) for a in
            (input_ids, hidden, cell, encoder_outputs, emb, U, W, Vw,
             Wih0, Whh0, bih0, bhh0, Wih1, Whh1, bih1, bhh1, out_w, out_b)]
    in_maps = _prep_inputs(*args)
    res = run_bass_kernel_spmd(nc, in_maps, core_ids=list(range(NCORES)),
                               trace=_trace)
    outs = res.results
    logits = np.concatenate([outs[c]["logits"] for c in range(NCORES)], 0)
    h_new = np.stack([
        np.concatenate([outs[c]["h0n"] for c in range(NCORES)], 0),
        np.concatenate([outs[c]["h1n"] for c in range(NCORES)], 0)])
    c_new = np.stack([
        np.concatenate([outs[c]["c0n"] for c in range(NCORES)], 0),
        np.concatenate([outs[c]["c1n"] for c in range(NCORES)], 0)])
    out = logits[:, None, :].astype(np.float32)
    kernel._last_results = res
    if int(np.asarray(matrix)):
        raise NotImplementedError("matrix=1 path not needed (reference uses 0)")
    return (out, h_new.astype(np.float32), c_new.astype(np.float32))


# revision 21
# speedup vs baseline: 1.1123x; 1.0216x over previous
"""Trainium2 Bass kernel for one attention-LSTM decoder step.

dims: B=512, S=64, H=1024, E=256, V=128, L=2, sharded data-parallel over
batch across 8 NeuronCores (64 batches/core). All matmuls run in fp16 with
fp32 PSUM accumulation; elementwise/softmax math in fp32.
"""

import sys

if "/opt/trn_rl_repo" not in sys.path:
    sys.path.insert(0, "/opt/trn_rl_repo")

import numpy as np

import concourse.bacc as bacc
import concourse.mybir as mybir
import concourse.tile as tile
from concourse.bass_utils import run_bass_kernel_spmd

B, S, H, E, V = 512, 64, 1024, 256, 128
NCORES = 8
BC = B // NCORES          # 64 batches per core
TOK = BC * S              # 4096 tokens per core
F = E + H                 # 1280 LSTM input features
G4 = 4 * H                # 4096 gate rows
F16 = mybir.dt.float16
F32 = mybir.dt.float32
AF = mybir.ActivationFunctionType
ALU = mybir.AluOpType
AX = mybir.AxisListType

_COMPILED = [None]


def _build():
    nc = bacc.Bacc("TRN2", target_bir_lowering=False, debug=False,
                   num_devices=NCORES)

    # ---- DRAM I/O ----
    d_encT = nc.dram_tensor("encT", [H, TOK], F16, kind="ExternalInput")
    d_encS = nc.dram_tensor("encS", [TOK, H], F16, kind="ExternalInput")
    d_U = nc.dram_tensor("Umat", [H, H], F16, kind="ExternalInput")
    d_W = nc.dram_tensor("Wmat", [H, H], F16, kind="ExternalInput")
    d_Vw = nc.dram_tensor("VwR", [128, 8], F16, kind="ExternalInput")
    d_hT0 = nc.dram_tensor("hT0", [128, 8 * BC], F16, kind="ExternalInput")
    d_hT1 = nc.dram_tensor("hT1", [128, 8 * BC], F16, kind="ExternalInput")
    d_ohT = nc.dram_tensor("onehotT", [V, BC], F16, kind="ExternalInput")
    d_emb = nc.dram_tensor("embW", [V, E], F16, kind="ExternalInput")
    d_oh64 = nc.dram_tensor("oh64", [BC, TOK], F16, kind="ExternalInput")
    d_I2 = nc.dram_tensor("I2", [128, 64], F32, kind="ExternalInput")
    d_I128 = nc.dram_tensor("I128", [128, 128], F16, kind="ExternalInput")
    d_ones1 = nc.dram_tensor("ones1", [1, BC], F16, kind="ExternalInput")
    d_wi0 = nc.dram_tensor("Wih0T", [F, G4], F16, kind="ExternalInput")
    d_wh0 = nc.dram_tensor("Whh0T", [H, G4], F16, kind="ExternalInput")
    d_wi1 = nc.dram_tensor("Wih1T", [H, G4], F16, kind="ExternalInput")
    d_wh1 = nc.dram_tensor("Whh1T", [H, G4], F16, kind="ExternalInput")
    d_b0 = nc.dram_tensor("bias0", [1, G4], F16, kind="ExternalInput")
    d_b1 = nc.dram_tensor("bias1", [1, G4], F16, kind="ExternalInput")
    d_c0 = nc.dram_tensor("cell0", [BC, H], F32, kind="ExternalInput")
    d_c1 = nc.dram_tensor("cell1", [BC, H], F32, kind="ExternalInput")
    d_owT = nc.dram_tensor("outWT", [H, V], F16, kind="ExternalInput")
    d_ob = nc.dram_tensor("outB", [1, V], F16, kind="ExternalInput")

    d_logits = nc.dram_tensor("logits", [BC, V], F32, kind="ExternalOutput")
    d_h0n = nc.dram_tensor("h0n", [BC, H], F32, kind="ExternalOutput")
    d_h1n = nc.dram_tensor("h1n", [BC, H], F32, kind="ExternalOutput")
    d_c0n = nc.dram_tensor("c0n", [BC, H], F32, kind="ExternalOutput")
    d_c1n = nc.dram_tensor("c1n", [BC, H], F32, kind="ExternalOutput")

    with tile.TileContext(nc) as tc:
        with (
            tc.tile_pool(name="const", bufs=1) as cpool,
            tc.tile_pool(name="stream", bufs=3) as stp,
            tc.tile_pool(name="tanh", bufs=2) as thp,
            tc.tile_pool(name="wls", bufs=3) as wpool,
            tc.tile_pool(name="mid", bufs=1) as mid,
            tc.tile_pool(name="psA", bufs=2, space="PSUM") as psA,
            tc.tile_pool(name="psE", bufs=2, space="PSUM") as psE,
            tc.tile_pool(name="psS", bufs=2, space="PSUM") as psS,
            tc.tile_pool(name="psT", bufs=2, space="PSUM") as psT,
        ):
            # ---------- resident constants ----------
            vw_sb = cpool.tile([128, 8], F16, tag="vw")
            nc.sync.dma_start(vw_sb[:], d_Vw[:])
            hT0_sb = cpool.tile([128, 8 * BC], F16, tag="hT0")
            nc.sync.dma_start(hT0_sb[:], d_hT0[:])
            hT1_sb = cpool.tile([128, 8 * BC], F16, tag="hT1")
            nc.sync.dma_start(hT1_sb[:], d_hT1[:])
            ohT_sb = cpool.tile([V, BC], F16, tag="ohT")
            nc.sync.dma_start(ohT_sb[:], d_ohT[:])
            emb_sb = cpool.tile([V, E], F16, tag="emb")
            nc.sync.dma_start(emb_sb[:], d_emb[:])
            ones1_sb = cpool.tile([1, BC], F16, tag="ones1")
            nc.sync.dma_start(ones1_sb[:], d_ones1[:])

            # ---------- phase A: t2 = h_top @ W  -> [BC, H] f16 ----------
            t2_sb = mid.tile([BC, H], F16, tag="t2")
            for n2 in range(2):
                ps = psS.tile([BC, 512], F32, tag="ps64", name="t2ps")
                wsl = stp.tile([128, 4096], F16, tag="big", name="wsl")
                nc.sync.dma_start(
                    wsl[:].rearrange("p (k c) -> p k c", c=512),
                    d_W.rearrange("(k p) h -> k p h", p=128)
                       [:, :, 512 * n2:512 * (n2 + 1)]
                       .transpose([1, 0, 2]))
                for k in range(8):
                    nc.tensor.matmul(
                        ps[:], hT1_sb[:, 64 * k:64 * (k + 1)],
                        wsl[:, 512 * k:512 * (k + 1)],
                        start=(k == 0), stop=(k == 7))
                nc.vector.tensor_copy(t2_sb[:, 512 * n2:512 * (n2 + 1)], ps[:])

            U_sb = [cpool.tile([128, H], F16, tag=f"U{k}", name=f"U{k}") for k in range(8)]
            for k in range(8):
                nc.scalar.dma_start(U_sb[k][:], d_U[128 * k:128 * (k + 1), :])
            # ---------- phase B: embedded^T -> xT blocks 0..1 ----------
            xT_sb = mid.tile([128, 10 * BC], F16, tag="xT")
            for et in range(2):
                ps = psT.tile([128, BC], F32, tag="pT", name="embps")
                nc.tensor.matmul(ps[:], emb_sb[:, 128 * et:128 * (et + 1)],
                                 ohT_sb[:], start=True, stop=True)
                nc.vector.tensor_copy(xT_sb[:, BC * et:BC * (et + 1)], ps[:])

            def load_et(n):
                t = stp.tile([128, 4096], F16, tag="big", name=f"et{n}")
                nc.sync.dma_start(
                    t[:].rearrange("p (k c) -> p k c", c=512),
                    d_encT.rearrange("(k p) t -> k p t", p=128)
                          [:, :, 512 * n:512 * (n + 1)]
                          .transpose([1, 0, 2]))
                return t

            def load_oh(n):
                t = stp.tile([BC, 512], F16, tag="oh", name=f"oh{n}")
                nc.sync.dma_start(t[:], d_oh64[:, 512 * n:512 * (n + 1)])
                return t

            pre_et = {n: load_et(n) for n in range(2)}
            pre_oh = {n: load_oh(n) for n in range(2)}
            def load_w(wd, n_k, n, eng):
                t = wpool.tile([128, n_k * 512], F16,
                               tag="wx" if n_k > 8 else "wh", name=f"w{n}")
                eng.dma_start(
                    t[:].rearrange("p (k c) -> p k c", c=512),
                    wd.rearrange("(k p) g -> k p g", p=128)
                      [:, :, 512 * n:512 * (n + 1)].transpose([1, 0, 2]))
                return t

            pre_wx0 = {n: load_w(d_wi0, 10, n, nc.scalar) for n in range(3)}
            pre_wh0 = {n: load_w(d_wh0, 8, n, nc.sync) for n in range(3)}

            # ---------- phase C: t1 + t2 -> tanh -> scores -> softmax -> ct ---
            # token tile n holds ALL 64 s-positions of batches 8n..8n+8, so
            # each tile's softmax + attention-context can be computed inline
            # against the already-resident encT tile (no second enc stream).
            ctT_sb = mid.tile([128, 512], F32, tag="ctT")  # [h-blk k][8n+b] cols
            for n in range(8):          # token tiles (512 tokens, 8 batches)
                et = pre_et.pop(n) if n in pre_et else load_et(n)
                oh64_t = pre_oh.pop(n) if n in pre_oh else load_oh(n)
                pe = psE.tile([1, 512], F32, tag="eps")
                for m in range(8):      # output-H tiles
                    pt = psA.tile([128, 512], F32, tag="t1ps")
                    for k in range(8):
                        nc.tensor.matmul(
                            pt[:], U_sb[k][:, 128 * m:128 * (m + 1)],
                            et[:, 512 * k:512 * (k + 1)],
                            start=(k == 0), stop=False)
                    # inject t2 broadcast over s:  lhsT=[64b,128h'] rhs=[64b,512tok]
                    nc.tensor.matmul(
                        pt[:], t2_sb[:, 128 * m:128 * (m + 1)],
                        oh64_t[:], start=False, stop=True)
                    th = thp.tile([128, 512], F16, tag="tanh")
                    nc.scalar.activation(th[:], pt[:], AF.Tanh)
                    nc.tensor.matmul(pe[:], vw_sb[:, m:m + 1], th[:],
                                     start=(m == 0), stop=(m == 7))
                # --- inline softmax over s for batches 8n..8n+8 ---
                er = mid.tile([1, 512], F32, tag="er", name="er", bufs=2)
                nc.vector.tensor_copy(er[:], pe[:])
                eb = mid.tile([8, S], F32, tag="eb", name="eb", bufs=2)
                nc.gpsimd.dma_start(
                    eb[:], er[0:1, :].rearrange("p (b s) -> p b s", b=8))
                mx = mid.tile([8, 1], F32, tag="mx", name="mx", bufs=2)
                nc.vector.tensor_reduce(mx[:], eb[:], axis=AX.X, op=ALU.max)
                negmx = mid.tile([8, 1], F32, tag="negmx", name="negmx", bufs=2)
                nc.vector.tensor_scalar_mul(negmx[:], mx[:], -1.0)
                pb = mid.tile([8, S], F32, tag="pb", name="pb", bufs=2)
                nc.scalar.activation(pb[:], eb[:], AF.Exp, bias=negmx[:])
                sm = mid.tile([8, 1], F32, tag="sm", name="sm", bufs=2)
                nc.vector.tensor_reduce(sm[:], pb[:], axis=AX.X, op=ALU.add)
                rinv = mid.tile([8, 1], F32, tag="rinv", name="rinv", bufs=2)
                nc.vector.reciprocal(rinv[:], sm[:])
                ab = mid.tile([8, S], F16, tag="ab", name="ab", bufs=2)
                nc.vector.tensor_scalar_mul(ab[:], pb[:], rinv[:])
                ar = mid.tile([1, 512], F16, tag="ar", name="ar", bufs=2)
                nc.gpsimd.dma_start(
                    ar[0:1, :].rearrange("p (b s) -> p b s", b=8), ab[:])
                af = mid.tile([128, 512], F16, tag="af", name="af", bufs=2)
                nc.gpsimd.partition_broadcast(af[:], ar[:])
                # --- ct^T columns for these 8 batches, per h-block k ---
                for k in range(8):
                    pr = thp.tile([128, 512], F16, tag="prod", name="pr", bufs=3)
                    nc.vector.tensor_tensor(
                        pr[:], et[:, 512 * k:512 * (k + 1)], af[:], ALU.mult)
                    nc.vector.tensor_reduce(
                        ctT_sb[:, 64 * k + 8 * n:64 * k + 8 * n + 8],
                        pr[:].rearrange("p (b s) -> p b s", b=8),
                        axis=AX.X, op=ALU.add)
            # ct^T blocked [h-blk][b] is exactly xT blocks 2..9
            nc.vector.tensor_copy(xT_sb[:, 2 * BC:10 * BC], ctT_sb[:])

            I128_sb = cpool.tile([128, 128], F16, tag="I128")
            nc.sync.dma_start(I128_sb[:], d_I128[:])
            c0_sb = cpool.tile([BC, H], F32, tag="c0")
            nc.sync.dma_start(c0_sb[:], d_c0[:])
            c1_sb = cpool.tile([BC, H], F32, tag="c1")
            nc.sync.dma_start(c1_sb[:], d_c1[:])
            owT_sb = [cpool.tile([128, V], F16, tag=f"ow{k}", name=f"ow{k}") for k in range(8)]
            for k in range(8):
                nc.sync.dma_start(owT_sb[k][:], d_owT[128 * k:128 * (k + 1), :])
            ob_sb = cpool.tile([1, V], F16, tag="ob")
            nc.sync.dma_start(ob_sb[:], d_ob[:])

            # ---------- phases G/H: two LSTM layers ----------
            def lstm_layer(xT, n_xk, wxd, whd, hT, bias_d, c_in,
                           d_hout, d_cout, hTout, lname, pre_wx, pre_wh):
                """xT: SBUF [128, n_xk*64] input^T blocks; whd/wxd DRAM weights;
                hT: SBUF [128, 8*64] prev-h^T blocks; returns nothing."""
                gates = mid.tile([BC, G4], F16, tag="gates", name="gates")
                bt = mid.tile([1, G4], F16, tag="btile", name="bt")
                nc.sync.dma_start(bt[:], bias_d[:])
                for n in range(8):
                    wx = pre_wx.pop(n) if n in pre_wx else \
                        load_w(wxd, n_xk, n, nc.scalar)
                    wh = pre_wh.pop(n) if n in pre_wh else \
                        load_w(whd, 8, n, nc.sync)
                    ps = psS.tile([BC, 512], F32, tag="ps64", name="gps")
                    for k in range(n_xk):
                        nc.tensor.matmul(ps[:], xT[:, 64 * k:64 * (k + 1)],
                                         wx[:, 512 * k:512 * (k + 1)],
                                         start=(k == 0), stop=False)
                    for k in range(8):
                        nc.tensor.matmul(ps[:], hT[:, 64 * k:64 * (k + 1)],
                                         wh[:, 512 * k:512 * (k + 1)],
                                         start=False, stop=False)
                    nc.tensor.matmul(ps[:], ones1_sb[:],
                                     bt[:, 512 * n:512 * (n + 1)],
                                     start=False, stop=True)
                    func = AF.Tanh if n in (4, 5) else AF.Sigmoid
                    nc.scalar.activation(gates[:, 512 * n:512 * (n + 1)],
                                         ps[:], func)
                # c2 = sig_f*c + sig_i*tanh_g ; h2 = sig_o*tanh(c2)
                tmp = mid.tile([BC, H], F32, tag="lstm_tmp", name="tmp")
                nc.vector.tensor_tensor(tmp[:], gates[:, 0:H],
                                        gates[:, 2 * H:3 * H], ALU.mult)
                c2 = mid.tile([BC, H], F32, tag="c2t", name="c2")
                nc.vector.tensor_tensor(c2[:], gates[:, H:2 * H], c_in[:],
                                        ALU.mult)
                nc.vector.tensor_tensor(c2[:], c2[:], tmp[:], ALU.add)
                nc.sync.dma_start(d_cout[:], c2[:])
                tc2 = mid.tile([BC, H], F32, tag="lstm_tmp", name="tc2")
                nc.scalar.activation(tc2[:], c2[:], AF.Tanh)
                h2 = mid.tile([BC, H], F32, tag="h2t", name="h2")
                nc.vector.tensor_tensor(h2[:], gates[:, 3 * H:4 * H], tc2[:],
                                        ALU.mult)
                nc.sync.dma_start(d_hout[:], h2[:])
                h2f = mid.tile([BC, H], F16, tag="lstm_h2f", name="h2f")
                nc.vector.tensor_copy(h2f[:], h2[:])
                for j in range(8):
                    pt = psT.tile([128, BC], F16, tag="pT", name="trps")
                    nc.tensor.transpose(pt[:], h2f[:, 128 * j:128 * (j + 1)],
                                        I128_sb[0:64, 0:64])
                    nc.vector.tensor_copy(hTout[:, BC * j:BC * (j + 1)], pt[:])

            h0T_sb = mid.tile([128, 8 * BC], F16, tag="h0T")
            lstm_layer(xT_sb, 10, d_wi0, d_wh0, hT0_sb, d_b0, c0_sb,
                       d_h0n, d_c0n, h0T_sb, "l0", pre_wx0, pre_wh0)
            h1T_sb = mid.tile([128, 8 * BC], F16, tag="h1T")
            lstm_layer(h0T_sb, 8, d_wi1, d_wh1, hT1_sb, d_b1, c1_sb,
                       d_h1n, d_c1n, h1T_sb, "l1", {}, {})

            # ---------- phase I: logits ----------
            pl = psS.tile([BC, V], F32, tag="ps64", name="lps")
            for k in range(8):
                nc.tensor.matmul(pl[:], h1T_sb[:, 64 * k:64 * (k + 1)],
                                 owT_sb[k][:], start=(k == 0), stop=False)
            nc.tensor.matmul(pl[:], ones1_sb[:], ob_sb[:],
                             start=False, stop=True)
            lo = mid.tile([BC, V], F32, tag="lo")
            nc.vector.tensor_copy(lo[:], pl[:])
            nc.sync.dma_start(d_logits[:], lo[:])

    nc.compile()
    return nc


def _prep_inputs(input_ids, hidden, cell, encoder_outputs, emb, U, W, Vw,
                 Wih0, Whh0, bih0, bhh0, Wih1, Whh1, bih1, bhh1,
                 out_w, out_b):
    f16 = np.float16
    # shared across cores
    U16 = np.ascontiguousarray(U.astype(f16))
    W16 = np.ascontiguousarray(W.astype(f16))
    VwR = np.ascontiguousarray(Vw.reshape(8, 128).T.astype(f16))  # [128,8]
    emb16 = np.ascontiguousarray(emb.astype(f16))
    oh64 = np.zeros((BC, TOK), f16)
    for b in range(BC):
        oh64[b, 64 * b:64 * (b + 1)] = 1.0
    I2 = np.zeros((128, 64), np.float32)
    I2[np.arange(128), np.arange(128) % 64] = 1.0
    I128 = np.eye(128, dtype=f16)
    ones1 = np.ones((1, BC), f16)
    Wih0T = np.ascontiguousarray(Wih0.T.astype(f16))
    Whh0T = np.ascontiguousarray(Whh0.T.astype(f16))
    Wih1T = np.ascontiguousarray(Wih1.T.astype(f16))
    Whh1T = np.ascontiguousarray(Whh1.T.astype(f16))
    b0 = np.ascontiguousarray((bih0 + bhh0)[None, :].astype(f16))
    b1 = np.ascontiguousarray((bih1 + bhh1)[None, :].astype(f16))
    owT = np.ascontiguousarray(out_w.T.astype(f16))
    ob = np.ascontiguousarray(out_b[None, :].astype(f16))

    def blocked_T(x):  # [BC,H] -> [128, 8*BC] (k-blocks of columns)
        t = np.ascontiguousarray(x.T)          # [H, BC]
        return np.ascontiguousarray(
            t.reshape(8, 128, BC).transpose(1, 0, 2).reshape(128, 8 * BC)
        ).astype(f16)

    ids = np.asarray(input_ids).reshape(B)
    in_maps = []
    for c in range(NCORES):
        bs = slice(BC * c, BC * (c + 1))
        enc_c = encoder_outputs[bs]                      # [BC, S, H]
        encT = np.ascontiguousarray(
            enc_c.reshape(TOK, H).T.astype(f16))         # [H, TOK] b-major
        encS = np.ascontiguousarray(
            enc_c.transpose(1, 0, 2).reshape(TOK, H).astype(f16))  # s-major
        ohT = np.zeros((V, BC), f16)
        ohT[ids[bs].astype(np.int64), np.arange(BC)] = 1.0
        in_maps.append({
            "encT": encT, "encS": encS, "Umat": U16, "Wmat": W16,
            "VwR": VwR,
            "hT0": blocked_T(hidden[0][bs]),
            "hT1": blocked_T(hidden[1][bs]),
            "onehotT": ohT, "embW": emb16, "oh64": oh64, "I2": I2,
            "I128": I128, "ones1": ones1,
            "Wih0T": Wih0T, "Whh0T": Whh0T, "Wih1T": Wih1T, "Whh1T": Whh1T,
            "bias0": b0, "bias1": b1,
            "cell0": np.ascontiguousarray(cell[0][bs], dtype=np.float32),
            "cell1": np.ascontiguousarray(cell[1][bs], dtype=np.float32),
            "outWT": owT, "outB": ob,
        })
    return in_maps


def kernel(input_ids, hidden, cell, encoder_outputs, emb, U, W, Vw,
           Wih0, Whh0, bih0, bhh0, Wih1, Whh1, bih1, bhh1,
           out_w, out_b, matrix=0, _trace=False):
    if _COMPILED[0] is None:
        _COMPILED[0] = _build()
    nc = _COMPILED[0]
    args = [np.asarray(a) for a in
            (input_ids, hidden, cell, encoder_outputs, emb, U, W, Vw,
             Wih0, Whh0, bih0, bhh0, Wih1, Whh1, bih1, bhh1, out_w, out_b)]
    in_maps = _prep_inputs(*args)
    res = run_bass_kernel_spmd(nc, in_maps, core_ids=list(range(NCORES)),
                               trace=_trace)
    outs = res.results
    logits = np.concatenate([outs[c]["logits"] for c in range(NCORES)], 0)
    h_new = np.stack([
        np.concatenate([outs[c]["h0n"] for c in range(NCORES)], 0),
        np.concatenate([outs[c]["h1n"] for c in range(NCORES)], 0)])
    c_new = np.stack([
        np.concatenate([outs[c]["c0n"] for c in range(NCORES)], 0),
        np.concatenate([outs[c]["c1n"] for c in range(NCORES)], 0)])
    out = logits[:, None, :].astype(np.float32)
    kernel._last_results = res
    if int(np.asarray(matrix)):
        raise NotImplementedError("matrix=1 path not needed (reference uses 0)")
    return (out, h_new.astype(np.float32), c_new.astype(np.float32))


# revision 23
# speedup vs baseline: 1.1476x; 1.0317x over previous
"""Trainium2 Bass kernel for one attention-LSTM decoder step.

dims: B=512, S=64, H=1024, E=256, V=128, L=2, sharded data-parallel over
batch across 8 NeuronCores (64 batches/core). All matmuls run in fp16 with
fp32 PSUM accumulation; elementwise/softmax math in fp32.
"""

import sys

if "/opt/trn_rl_repo" not in sys.path:
    sys.path.insert(0, "/opt/trn_rl_repo")

import numpy as np

import concourse.bacc as bacc
import concourse.mybir as mybir
import concourse.tile as tile
from concourse.bass_utils import run_bass_kernel_spmd

B, S, H, E, V = 512, 64, 1024, 256, 128
NCORES = 8
BC = B // NCORES          # 64 batches per core
TOK = BC * S              # 4096 tokens per core
F = E + H                 # 1280 LSTM input features
G4 = 4 * H                # 4096 gate rows
F16 = mybir.dt.float16
F32 = mybir.dt.float32
AF = mybir.ActivationFunctionType
ALU = mybir.AluOpType
AX = mybir.AxisListType

_COMPILED = [None]


def _build():
    nc = bacc.Bacc("TRN2", target_bir_lowering=False, debug=False,
                   num_devices=NCORES)

    # ---- DRAM I/O ----
    d_encT = nc.dram_tensor("encT", [H, TOK], F16, kind="ExternalInput")
    d_U = nc.dram_tensor("Umat", [H, H], F16, kind="ExternalInput")
    d_W = nc.dram_tensor("Wmat", [H, H], F16, kind="ExternalInput")
    d_Vw = nc.dram_tensor("VwR", [128, 8], F16, kind="ExternalInput")
    d_hT0 = nc.dram_tensor("hT0", [128, 8 * BC], F16, kind="ExternalInput")
    d_hT1 = nc.dram_tensor("hT1", [128, 8 * BC], F16, kind="ExternalInput")
    d_ohT = nc.dram_tensor("onehotT", [V, BC], F16, kind="ExternalInput")
    d_emb = nc.dram_tensor("embW", [V, E], F16, kind="ExternalInput")
    d_oh64 = nc.dram_tensor("oh64", [BC, TOK], F16, kind="ExternalInput")
    d_I128 = nc.dram_tensor("I128", [128, 128], F16, kind="ExternalInput")
    d_ones1 = nc.dram_tensor("ones1", [1, BC], F16, kind="ExternalInput")
    d_wi0 = nc.dram_tensor("Wih0T", [F, G4], F16, kind="ExternalInput")
    d_wh0 = nc.dram_tensor("Whh0T", [H, G4], F16, kind="ExternalInput")
    d_wi1 = nc.dram_tensor("Wih1T", [H, G4], F16, kind="ExternalInput")
    d_wh1 = nc.dram_tensor("Whh1T", [H, G4], F16, kind="ExternalInput")
    d_b0 = nc.dram_tensor("bias0", [1, G4], F16, kind="ExternalInput")
    d_b1 = nc.dram_tensor("bias1", [1, G4], F16, kind="ExternalInput")
    d_c0 = nc.dram_tensor("cell0", [BC, H], F32, kind="ExternalInput")
    d_c1 = nc.dram_tensor("cell1", [BC, H], F32, kind="ExternalInput")
    d_owT = nc.dram_tensor("outWT", [H, V], F16, kind="ExternalInput")
    d_ob = nc.dram_tensor("outB", [1, V], F16, kind="ExternalInput")

    d_logits = nc.dram_tensor("logits", [BC, V], F32, kind="ExternalOutput")
    d_h0n = nc.dram_tensor("h0n", [BC, H], F32, kind="ExternalOutput")
    d_h1n = nc.dram_tensor("h1n", [BC, H], F32, kind="ExternalOutput")
    d_c0n = nc.dram_tensor("c0n", [BC, H], F32, kind="ExternalOutput")
    d_c1n = nc.dram_tensor("c1n", [BC, H], F32, kind="ExternalOutput")

    with tile.TileContext(nc) as tc:
        with (
            tc.tile_pool(name="const", bufs=1) as cpool,
            tc.tile_pool(name="stream", bufs=3) as stp,
            tc.tile_pool(name="tanh", bufs=2) as thp,
            tc.tile_pool(name="wls", bufs=3) as wpool,
            tc.tile_pool(name="mid", bufs=1) as mid,
            tc.tile_pool(name="psA", bufs=2, space="PSUM") as psA,
            tc.tile_pool(name="psE", bufs=2, space="PSUM") as psE,
            tc.tile_pool(name="psS", bufs=2, space="PSUM") as psS,
            tc.tile_pool(name="psT", bufs=2, space="PSUM") as psT,
        ):
            # ---------- resident constants ----------
            vw_sb = cpool.tile([128, 8], F16, tag="vw")
            nc.sync.dma_start(vw_sb[:], d_Vw[:])
            hT0_sb = cpool.tile([128, 8 * BC], F16, tag="hT0")
            nc.sync.dma_start(hT0_sb[:], d_hT0[:])
            hT1_sb = cpool.tile([128, 8 * BC], F16, tag="hT1")
            nc.sync.dma_start(hT1_sb[:], d_hT1[:])
            ohT_sb = cpool.tile([V, BC], F16, tag="ohT")
            nc.sync.dma_start(ohT_sb[:], d_ohT[:])
            emb_sb = cpool.tile([V, E], F16, tag="emb")
            nc.sync.dma_start(emb_sb[:], d_emb[:])
            ones1_sb = cpool.tile([1, BC], F16, tag="ones1")
            nc.sync.dma_start(ones1_sb[:], d_ones1[:])

            # ---------- phase A: t2 = h_top @ W  -> [BC, H] f16 ----------
            t2_sb = mid.tile([BC, H], F16, tag="t2")
            for n2 in range(2):
                ps = psS.tile([BC, 512], F32, tag="ps64", name="t2ps")
                wsl = stp.tile([128, 4096], F16, tag="big", name="wsl")
                nc.sync.dma_start(
                    wsl[:].rearrange("p (k c) -> p k c", c=512),
                    d_W.rearrange("(k p) h -> k p h", p=128)
                       [:, :, 512 * n2:512 * (n2 + 1)]
                       .transpose([1, 0, 2]))
                for k in range(8):
                    nc.tensor.matmul(
                        ps[:], hT1_sb[:, 64 * k:64 * (k + 1)],
                        wsl[:, 512 * k:512 * (k + 1)],
                        start=(k == 0), stop=(k == 7))
                nc.vector.tensor_copy(t2_sb[:, 512 * n2:512 * (n2 + 1)], ps[:])

            U_sb = [cpool.tile([128, H], F16, tag=f"U{k}", name=f"U{k}") for k in range(8)]
            for k in range(8):
                nc.scalar.dma_start(U_sb[k][:], d_U[128 * k:128 * (k + 1), :])
            # ---------- phase B: embedded^T -> xT blocks 0..1 ----------
            xT_sb = mid.tile([128, 10 * BC], F16, tag="xT")
            for et in range(2):
                ps = psT.tile([128, BC], F32, tag="pT", name="embps")
                nc.tensor.matmul(ps[:], emb_sb[:, 128 * et:128 * (et + 1)],
                                 ohT_sb[:], start=True, stop=True)
                nc.vector.tensor_copy(xT_sb[:, BC * et:BC * (et + 1)], ps[:])

            def load_et(n):
                t = stp.tile([128, 4096], F16, tag="big", name=f"et{n}")
                nc.sync.dma_start(
                    t[:].rearrange("p (k c) -> p k c", c=512),
                    d_encT.rearrange("(k p) t -> k p t", p=128)
                          [:, :, 512 * n:512 * (n + 1)]
                          .transpose([1, 0, 2]))
                return t

            def load_oh(n):
                t = stp.tile([BC, 512], F16, tag="oh", name=f"oh{n}")
                nc.sync.dma_start(t[:], d_oh64[:, 512 * n:512 * (n + 1)])
                return t

            pre_et = {n: load_et(n) for n in range(2)}
            pre_oh = {n: load_oh(n) for n in range(2)}
            def load_w(wd, n_k, n, eng):
                t = wpool.tile([128, n_k * 512], F16,
                               tag="wx" if n_k > 8 else "wh", name=f"w{n}")
                eng.dma_start(
                    t[:].rearrange("p (k c) -> p k c", c=512),
                    wd.rearrange("(k p) g -> k p g", p=128)
                      [:, :, 512 * n:512 * (n + 1)].transpose([1, 0, 2]))
                return t

            pre_wx0 = {n: load_w(d_wi0, 10, n, nc.scalar) for n in range(3)}
            pre_wh0 = {n: load_w(d_wh0, 8, n, nc.sync) for n in range(3)}

            # ---------- phase C: t1 + t2 -> tanh -> scores -> softmax -> ct ---
            # token tile n holds ALL 64 s-positions of batches 8n..8n+8, so
            # each tile's softmax + attention-context can be computed inline
            # against the already-resident encT tile (no second enc stream).
            ctT_sb = mid.tile([128, 512], F32, tag="ctT")  # [h-blk k][8n+b] cols
            for n in range(8):          # token tiles (512 tokens, 8 batches)
                et = pre_et.pop(n) if n in pre_et else load_et(n)
                oh64_t = pre_oh.pop(n) if n in pre_oh else load_oh(n)
                pe = psE.tile([1, 512], F32, tag="eps")
                for m in range(8):      # output-H tiles
                    pt = psA.tile([128, 512], F32, tag="t1ps")
                    for k in range(8):
                        nc.tensor.matmul(
                            pt[:], U_sb[k][:, 128 * m:128 * (m + 1)],
                            et[:, 512 * k:512 * (k + 1)],
                            start=(k == 0), stop=False)
                    # inject t2 broadcast over s:  lhsT=[64b,128h'] rhs=[64b,512tok]
                    nc.tensor.matmul(
                        pt[:], t2_sb[:, 128 * m:128 * (m + 1)],
                        oh64_t[:], start=False, stop=True)
                    th = thp.tile([128, 512], F16, tag="tanh")
                    nc.scalar.activation(th[:], pt[:], AF.Tanh)
                    nc.tensor.matmul(pe[:], vw_sb[:, m:m + 1], th[:],
                                     start=(m == 0), stop=(m == 7))
                # --- inline softmax over s for batches 8n..8n+8 ---
                er = mid.tile([1, 512], F32, tag="er", name="er", bufs=2)
                nc.vector.tensor_copy(er[:], pe[:])
                eb = mid.tile([8, S], F32, tag="eb", name="eb", bufs=2)
                nc.gpsimd.dma_start(
                    eb[:], er[0:1, :].rearrange("p (b s) -> p b s", b=8))
                mx = mid.tile([8, 1], F32, tag="mx", name="mx", bufs=2)
                nc.vector.tensor_reduce(mx[:], eb[:], axis=AX.X, op=ALU.max)
                negmx = mid.tile([8, 1], F32, tag="negmx", name="negmx", bufs=2)
                nc.vector.tensor_scalar_mul(negmx[:], mx[:], -1.0)
                pb = mid.tile([8, S], F32, tag="pb", name="pb", bufs=2)
                nc.scalar.activation(pb[:], eb[:], AF.Exp, bias=negmx[:])
                sm = mid.tile([8, 1], F32, tag="sm", name="sm", bufs=2)
                nc.vector.tensor_reduce(sm[:], pb[:], axis=AX.X, op=ALU.add)
                rinv = mid.tile([8, 1], F32, tag="rinv", name="rinv", bufs=2)
                nc.vector.reciprocal(rinv[:], sm[:])
                ab = mid.tile([8, S], F16, tag="ab", name="ab", bufs=2)
                nc.vector.tensor_scalar_mul(ab[:], pb[:], rinv[:])
                ar = mid.tile([1, 512], F16, tag="ar", name="ar", bufs=2)
                nc.gpsimd.dma_start(
                    ar[0:1, :].rearrange("p (b s) -> p b s", b=8), ab[:])
                af = mid.tile([128, 512], F16, tag="af", name="af", bufs=2)
                nc.gpsimd.partition_broadcast(af[:], ar[:])
                # --- ct^T columns for these 8 batches, per h-block k ---
                for k in range(8):
                    pr = thp.tile([128, 512], F16, tag="prod", name="pr", bufs=3)
                    nc.vector.tensor_tensor(
                        pr[:], et[:, 512 * k:512 * (k + 1)], af[:], ALU.mult)
                    nc.vector.tensor_reduce(
                        ctT_sb[:, 64 * k + 8 * n:64 * k + 8 * n + 8],
                        pr[:].rearrange("p (b s) -> p b s", b=8),
                        axis=AX.X, op=ALU.add)
            # ct^T blocked [h-blk][b] is exactly xT blocks 2..9
            nc.vector.tensor_copy(xT_sb[:, 2 * BC:10 * BC], ctT_sb[:])

            I128_sb = cpool.tile([128, 128], F16, tag="I128")
            nc.sync.dma_start(I128_sb[:], d_I128[:])
            c0_sb = cpool.tile([BC, H], F32, tag="c0")
            nc.sync.dma_start(c0_sb[:], d_c0[:])
            c1_sb = cpool.tile([BC, H], F32, tag="c1")
            nc.sync.dma_start(c1_sb[:], d_c1[:])
            owT_sb = [cpool.tile([128, V], F16, tag=f"ow{k}", name=f"ow{k}") for k in range(8)]
            for k in range(8):
                nc.sync.dma_start(owT_sb[k][:], d_owT[128 * k:128 * (k + 1), :])
            ob_sb = cpool.tile([1, V], F16, tag="ob")
            nc.sync.dma_start(ob_sb[:], d_ob[:])

            # ---------- phases G/H: two LSTM layers ----------
            def lstm_layer(xT, n_xk, wxd, whd, hT, bias_d, c_in,
                           d_hout, d_cout, hTout, lname, pre_wx, pre_wh):
                """xT: SBUF [128, n_xk*64] input^T blocks; whd/wxd DRAM weights;
                hT: SBUF [128, 8*64] prev-h^T blocks; returns nothing."""
                gates = mid.tile([BC, G4], F16, tag="gates", name="gates")
                bt = mid.tile([1, G4], F16, tag="btile", name="bt")
                nc.sync.dma_start(bt[:], bias_d[:])
                for n in range(8):
                    wx = pre_wx.pop(n) if n in pre_wx else \
                        load_w(wxd, n_xk, n, nc.scalar)
                    wh = pre_wh.pop(n) if n in pre_wh else \
                        load_w(whd, 8, n, nc.sync)
                    ps = psS.tile([BC, 512], F32, tag="ps64", name="gps")
                    for k in range(n_xk):
                        nc.tensor.matmul(ps[:], xT[:, 64 * k:64 * (k + 1)],
                                         wx[:, 512 * k:512 * (k + 1)],
                                         start=(k == 0), stop=False)
                    for k in range(8):
                        nc.tensor.matmul(ps[:], hT[:, 64 * k:64 * (k + 1)],
                                         wh[:, 512 * k:512 * (k + 1)],
                                         start=False, stop=False)
                    nc.tensor.matmul(ps[:], ones1_sb[:],
                                     bt[:, 512 * n:512 * (n + 1)],
                                     start=False, stop=True)
                    func = AF.Tanh if n in (4, 5) else AF.Sigmoid
                    nc.scalar.activation(gates[:, 512 * n:512 * (n + 1)],
                                         ps[:], func)
                # c2 = sig_f*c + sig_i*tanh_g ; h2 = sig_o*tanh(c2)
                tmp = mid.tile([BC, H], F32, tag="lstm_tmp", name="tmp")
                nc.vector.tensor_tensor(tmp[:], gates[:, 0:H],
                                        gates[:, 2 * H:3 * H], ALU.mult)
                c2 = mid.tile([BC, H], F32, tag="c2t", name="c2")
                nc.vector.tensor_tensor(c2[:], gates[:, H:2 * H], c_in[:],
                                        ALU.mult)
                nc.vector.tensor_tensor(c2[:], c2[:], tmp[:], ALU.add)
                nc.sync.dma_start(d_cout[:], c2[:])
                tc2 = mid.tile([BC, H], F32, tag="lstm_tmp", name="tc2")
                nc.scalar.activation(tc2[:], c2[:], AF.Tanh)
                h2 = mid.tile([BC, H], F32, tag="h2t", name="h2")
                nc.vector.tensor_tensor(h2[:], gates[:, 3 * H:4 * H], tc2[:],
                                        ALU.mult)
                nc.sync.dma_start(d_hout[:], h2[:])
                h2f = mid.tile([BC, H], F16, tag="lstm_h2f", name="h2f")
                nc.vector.tensor_copy(h2f[:], h2[:])
                for j in range(8):
                    pt = psT.tile([128, BC], F16, tag="pT", name="trps")
                    nc.tensor.transpose(pt[:], h2f[:, 128 * j:128 * (j + 1)],
                                        I128_sb[0:64, 0:64])
                    nc.vector.tensor_copy(hTout[:, BC * j:BC * (j + 1)], pt[:])

            h0T_sb = mid.tile([128, 8 * BC], F16, tag="h0T")
            lstm_layer(xT_sb, 10, d_wi0, d_wh0, hT0_sb, d_b0, c0_sb,
                       d_h0n, d_c0n, h0T_sb, "l0", pre_wx0, pre_wh0)
            h1T_sb = mid.tile([128, 8 * BC], F16, tag="h1T")
            lstm_layer(h0T_sb, 8, d_wi1, d_wh1, hT1_sb, d_b1, c1_sb,
                       d_h1n, d_c1n, h1T_sb, "l1", {}, {})

            # ---------- phase I: logits ----------
            pl = psS.tile([BC, V], F32, tag="ps64", name="lps")
            for k in range(8):
                nc.tensor.matmul(pl[:], h1T_sb[:, 64 * k:64 * (k + 1)],
                                 owT_sb[k][:], start=(k == 0), stop=False)
            nc.tensor.matmul(pl[:], ones1_sb[:], ob_sb[:],
                             start=False, stop=True)
            lo = mid.tile([BC, V], F32, tag="lo")
            nc.vector.tensor_copy(lo[:], pl[:])
            nc.sync.dma_start(d_logits[:], lo[:])

    nc.compile()
    return nc


def _prep_inputs(input_ids, hidden, cell, encoder_outputs, emb, U, W, Vw,
                 Wih0, Whh0, bih0, bhh0, Wih1, Whh1, bih1, bhh1,
                 out_w, out_b):
    f16 = np.float16
    # shared across cores
    U16 = np.ascontiguousarray(U.astype(f16))
    W16 = np.ascontiguousarray(W.astype(f16))
    VwR = np.ascontiguousarray(Vw.reshape(8, 128).T.astype(f16))  # [128,8]
    emb16 = np.ascontiguousarray(emb.astype(f16))
    oh64 = np.zeros((BC, TOK), f16)
    for b in range(BC):
        oh64[b, 64 * b:64 * (b + 1)] = 1.0
    I128 = np.eye(128, dtype=f16)
    ones1 = np.ones((1, BC), f16)
    Wih0T = np.ascontiguousarray(Wih0.T.astype(f16))
    Whh0T = np.ascontiguousarray(Whh0.T.astype(f16))
    Wih1T = np.ascontiguousarray(Wih1.T.astype(f16))
    Whh1T = np.ascontiguousarray(Whh1.T.astype(f16))
    b0 = np.ascontiguousarray((bih0 + bhh0)[None, :].astype(f16))
    b1 = np.ascontiguousarray((bih1 + bhh1)[None, :].astype(f16))
    owT = np.ascontiguousarray(out_w.T.astype(f16))
    ob = np.ascontiguousarray(out_b[None, :].astype(f16))

    def blocked_T(x):  # [BC,H] -> [128, 8*BC] (k-blocks of columns)
        t = np.ascontiguousarray(x.T)          # [H, BC]
        return np.ascontiguousarray(
            t.reshape(8, 128, BC).transpose(1, 0, 2).reshape(128, 8 * BC)
        ).astype(f16)

    ids = np.asarray(input_ids).reshape(B)
    in_maps = []
    for c in range(NCORES):
        bs = slice(BC * c, BC * (c + 1))
        enc_c = encoder_outputs[bs]                      # [BC, S, H]
        encT = np.ascontiguousarray(
            enc_c.reshape(TOK, H).T.astype(f16))         # [H, TOK] b-major
        ohT = np.zeros((V, BC), f16)
        ohT[ids[bs].astype(np.int64), np.arange(BC)] = 1.0
        in_maps.append({
            "encT": encT, "Umat": U16, "Wmat": W16,
            "VwR": VwR,
            "hT0": blocked_T(hidden[0][bs]),
            "hT1": blocked_T(hidden[1][bs]),
            "onehotT": ohT, "embW": emb16, "oh64": oh64,
            "I128": I128, "ones1": ones1,
            "Wih0T": Wih0T, "Whh0T": Whh0T, "Wih1T": Wih1T, "Whh1T": Whh1T,
            "bias0": b0, "bias1": b1,
            "cell0": np.ascontiguousarray(cell[0][bs], dtype=np.float32),
            "cell1": np.ascontiguousarray(cell[1][bs], dtype=np.float32),
            "outWT": owT, "outB": ob,
        })
    return in_maps


def kernel(input_ids, hidden, cell, encoder_outputs, emb, U, W, Vw,
           Wih0, Whh0, bih0, bhh0, Wih1, Whh1, bih1, bhh1,
           out_w, out_b, matrix=0, _trace=False):
    if _COMPILED[0] is None:
        _COMPILED[0] = _build()
    nc = _COMPILED[0]
    args = [np.asarray(a) for a in
            (input_ids, hidden, cell, encoder_outputs, emb, U, W, Vw,
             Wih0, Whh0, bih0, bhh0, Wih1, Whh1, bih1, bhh1, out_w, out_b)]
    in_maps = _prep_inputs(*args)
    res = run_bass_kernel_spmd(nc, in_maps, core_ids=list(range(NCORES)),
                               trace=_trace)
    outs = res.results
    logits = np.concatenate([outs[c]["logits"] for c in range(NCORES)], 0)
    h_new = np.stack([
        np.concatenate([outs[c]["h0n"] for c in range(NCORES)], 0),
        np.concatenate([outs[c]["h1n"] for c in range(NCORES)], 0)])
    c_new = np.stack([
        np.concatenate([outs[c]["c0n"] for c in range(NCORES)], 0),
        np.concatenate([outs[c]["c1n"] for c in range(NCORES)], 0)])
    out = logits[:, None, :].astype(np.float32)
    kernel._last_results = res
    if int(np.asarray(matrix)):
        raise NotImplementedError("matrix=1 path not needed (reference uses 0)")
    return (out, h_new.astype(np.float32), c_new.astype(np.float32))


# revision 24
# speedup vs baseline: 1.1543x; 1.0059x over previous
"""Trainium2 Bass kernel for one attention-LSTM decoder step.

dims: B=512, S=64, H=1024, E=256, V=128, L=2, sharded data-parallel over
batch across 8 NeuronCores (64 batches/core). All matmuls run in fp16 with
fp32 PSUM accumulation; elementwise/softmax math in fp32.
"""

import sys

if "/opt/trn_rl_repo" not in sys.path:
    sys.path.insert(0, "/opt/trn_rl_repo")

import numpy as np

import concourse.bacc as bacc
import concourse.mybir as mybir
import concourse.tile as tile
from concourse.bass_utils import run_bass_kernel_spmd

B, S, H, E, V = 512, 64, 1024, 256, 128
NCORES = 8
BC = B // NCORES          # 64 batches per core
TOK = BC * S              # 4096 tokens per core
F = E + H                 # 1280 LSTM input features
G4 = 4 * H                # 4096 gate rows
F16 = mybir.dt.float16
F32 = mybir.dt.float32
AF = mybir.ActivationFunctionType
ALU = mybir.AluOpType
AX = mybir.AxisListType

_COMPILED = [None]


def _build():
    nc = bacc.Bacc("TRN2", target_bir_lowering=False, debug=False,
                   num_devices=NCORES)

    # ---- DRAM I/O ----
    d_encT = nc.dram_tensor("encT", [H, TOK], F16, kind="ExternalInput")
    d_U = nc.dram_tensor("Umat", [H, H], F16, kind="ExternalInput")
    d_W = nc.dram_tensor("Wmat", [H, H], F16, kind="ExternalInput")
    d_Vw = nc.dram_tensor("VwR", [128, 8], F16, kind="ExternalInput")
    d_hT0 = nc.dram_tensor("hT0", [128, 8 * BC], F16, kind="ExternalInput")
    d_hT1 = nc.dram_tensor("hT1", [128, 8 * BC], F16, kind="ExternalInput")
    d_ohT = nc.dram_tensor("onehotT", [V, BC], F16, kind="ExternalInput")
    d_emb = nc.dram_tensor("embW", [V, E], F16, kind="ExternalInput")
    d_oh64 = nc.dram_tensor("oh64", [BC, TOK], F16, kind="ExternalInput")
    d_I128 = nc.dram_tensor("I128", [128, 128], F16, kind="ExternalInput")
    d_ones1 = nc.dram_tensor("ones1", [1, BC], F16, kind="ExternalInput")
    d_wi0 = nc.dram_tensor("Wih0T", [F, G4], F16, kind="ExternalInput")
    d_wh0 = nc.dram_tensor("Whh0T", [H, G4], F16, kind="ExternalInput")
    d_wi1 = nc.dram_tensor("Wih1T", [H, G4], F16, kind="ExternalInput")
    d_wh1 = nc.dram_tensor("Whh1T", [H, G4], F16, kind="ExternalInput")
    d_b0 = nc.dram_tensor("bias0", [1, G4], F16, kind="ExternalInput")
    d_b1 = nc.dram_tensor("bias1", [1, G4], F16, kind="ExternalInput")
    d_c0 = nc.dram_tensor("cell0", [BC, H], F32, kind="ExternalInput")
    d_c1 = nc.dram_tensor("cell1", [BC, H], F32, kind="ExternalInput")
    d_owT = nc.dram_tensor("outWT", [H, V], F16, kind="ExternalInput")
    d_ob = nc.dram_tensor("outB", [1, V], F16, kind="ExternalInput")

    d_logits = nc.dram_tensor("logits", [BC, V], F32, kind="ExternalOutput")
    d_h0n = nc.dram_tensor("h0n", [BC, H], F32, kind="ExternalOutput")
    d_h1n = nc.dram_tensor("h1n", [BC, H], F32, kind="ExternalOutput")
    d_c0n = nc.dram_tensor("c0n", [BC, H], F32, kind="ExternalOutput")
    d_c1n = nc.dram_tensor("c1n", [BC, H], F32, kind="ExternalOutput")

    with tile.TileContext(nc) as tc:
        with (
            tc.tile_pool(name="const", bufs=1) as cpool,
            tc.tile_pool(name="stream", bufs=3) as stp,
            tc.tile_pool(name="tanh", bufs=3) as thp,
            tc.tile_pool(name="wls", bufs=3) as wpool,
            tc.tile_pool(name="mid", bufs=1) as mid,
            tc.tile_pool(name="psA", bufs=3, space="PSUM") as psA,
            tc.tile_pool(name="psE", bufs=1, space="PSUM") as psE,
            tc.tile_pool(name="psS", bufs=2, space="PSUM") as psS,
            tc.tile_pool(name="psT", bufs=2, space="PSUM") as psT,
        ):
            # ---------- resident constants ----------
            vw_sb = cpool.tile([128, 8], F16, tag="vw")
            nc.sync.dma_start(vw_sb[:], d_Vw[:])
            hT0_sb = cpool.tile([128, 8 * BC], F16, tag="hT0")
            nc.sync.dma_start(hT0_sb[:], d_hT0[:])
            hT1_sb = cpool.tile([128, 8 * BC], F16, tag="hT1")
            nc.sync.dma_start(hT1_sb[:], d_hT1[:])
            ohT_sb = cpool.tile([V, BC], F16, tag="ohT")
            nc.sync.dma_start(ohT_sb[:], d_ohT[:])
            emb_sb = cpool.tile([V, E], F16, tag="emb")
            nc.sync.dma_start(emb_sb[:], d_emb[:])
            ones1_sb = cpool.tile([1, BC], F16, tag="ones1")
            nc.sync.dma_start(ones1_sb[:], d_ones1[:])

            # ---------- phase A: t2 = h_top @ W  -> [BC, H] f16 ----------
            t2_sb = mid.tile([BC, H], F16, tag="t2")
            for n2 in range(2):
                ps = psS.tile([BC, 512], F32, tag="ps64", name="t2ps")
                wsl = stp.tile([128, 4096], F16, tag="big", name="wsl")
                nc.sync.dma_start(
                    wsl[:].rearrange("p (k c) -> p k c", c=512),
                    d_W.rearrange("(k p) h -> k p h", p=128)
                       [:, :, 512 * n2:512 * (n2 + 1)]
                       .transpose([1, 0, 2]))
                for k in range(8):
                    nc.tensor.matmul(
                        ps[:], hT1_sb[:, 64 * k:64 * (k + 1)],
                        wsl[:, 512 * k:512 * (k + 1)],
                        start=(k == 0), stop=(k == 7))
                nc.vector.tensor_copy(t2_sb[:, 512 * n2:512 * (n2 + 1)], ps[:])

            U_sb = [cpool.tile([128, H], F16, tag=f"U{k}", name=f"U{k}") for k in range(8)]
            for k in range(8):
                nc.scalar.dma_start(U_sb[k][:], d_U[128 * k:128 * (k + 1), :])
            # ---------- phase B: embedded^T -> xT blocks 0..1 ----------
            xT_sb = mid.tile([128, 10 * BC], F16, tag="xT")
            for et in range(2):
                ps = psT.tile([128, BC], F32, tag="pT", name="embps")
                nc.tensor.matmul(ps[:], emb_sb[:, 128 * et:128 * (et + 1)],
                                 ohT_sb[:], start=True, stop=True)
                nc.vector.tensor_copy(xT_sb[:, BC * et:BC * (et + 1)], ps[:])

            def load_et(n):
                t = stp.tile([128, 4096], F16, tag="big", name=f"et{n}")
                nc.sync.dma_start(
                    t[:].rearrange("p (k c) -> p k c", c=512),
                    d_encT.rearrange("(k p) t -> k p t", p=128)
                          [:, :, 512 * n:512 * (n + 1)]
                          .transpose([1, 0, 2]))
                return t

            def load_oh(n):
                t = stp.tile([BC, 512], F16, tag="oh", name=f"oh{n}")
                nc.sync.dma_start(t[:], d_oh64[:, 512 * n:512 * (n + 1)])
                return t

            pre_et = {n: load_et(n) for n in range(2)}
            pre_oh = {n: load_oh(n) for n in range(2)}
            def load_w(wd, n_k, n, eng):
                t = wpool.tile([128, n_k * 512], F16,
                               tag="wx" if n_k > 8 else "wh", name=f"w{n}")
                eng.dma_start(
                    t[:].rearrange("p (k c) -> p k c", c=512),
                    wd.rearrange("(k p) g -> k p g", p=128)
                      [:, :, 512 * n:512 * (n + 1)].transpose([1, 0, 2]))
                return t

            pre_wx0 = {n: load_w(d_wi0, 10, n, nc.scalar) for n in range(3)}
            pre_wh0 = {n: load_w(d_wh0, 8, n, nc.sync) for n in range(3)}

            # ---------- phase C: t1 + t2 -> tanh -> scores -> softmax -> ct ---
            # token tile n holds ALL 64 s-positions of batches 8n..8n+8, so
            # each tile's softmax + attention-context can be computed inline
            # against the already-resident encT tile (no second enc stream).
            ctT_sb = mid.tile([128, 512], F32, tag="ctT")  # [h-blk k][8n+b] cols
            for n in range(8):          # token tiles (512 tokens, 8 batches)
                et = pre_et.pop(n) if n in pre_et else load_et(n)
                oh64_t = pre_oh.pop(n) if n in pre_oh else load_oh(n)
                pe = psE.tile([1, 512], F32, tag="eps")
                for m in range(8):      # output-H tiles
                    pt = psA.tile([128, 512], F32, tag="t1ps")
                    for k in range(8):
                        nc.tensor.matmul(
                            pt[:], U_sb[k][:, 128 * m:128 * (m + 1)],
                            et[:, 512 * k:512 * (k + 1)],
                            start=(k == 0), stop=False)
                    # inject t2 broadcast over s:  lhsT=[64b,128h'] rhs=[64b,512tok]
                    nc.tensor.matmul(
                        pt[:], t2_sb[:, 128 * m:128 * (m + 1)],
                        oh64_t[:], start=False, stop=True)
                    th = thp.tile([128, 512], F16, tag="tanh")
                    nc.scalar.activation(th[:], pt[:], AF.Tanh)
                    nc.tensor.matmul(pe[:], vw_sb[:, m:m + 1], th[:],
                                     start=(m == 0), stop=(m == 7))
                # --- inline softmax over s for batches 8n..8n+8 ---
                er = mid.tile([1, 512], F32, tag="er", name="er", bufs=2)
                nc.vector.tensor_copy(er[:], pe[:])
                eb = mid.tile([8, S], F32, tag="eb", name="eb", bufs=2)
                nc.gpsimd.dma_start(
                    eb[:], er[0:1, :].rearrange("p (b s) -> p b s", b=8))
                mx = mid.tile([8, 1], F32, tag="mx", name="mx", bufs=2)
                nc.vector.tensor_reduce(mx[:], eb[:], axis=AX.X, op=ALU.max)
                negmx = mid.tile([8, 1], F32, tag="negmx", name="negmx", bufs=2)
                nc.vector.tensor_scalar_mul(negmx[:], mx[:], -1.0)
                pb = mid.tile([8, S], F32, tag="pb", name="pb", bufs=2)
                nc.scalar.activation(pb[:], eb[:], AF.Exp, bias=negmx[:])
                sm = mid.tile([8, 1], F32, tag="sm", name="sm", bufs=2)
                nc.vector.tensor_reduce(sm[:], pb[:], axis=AX.X, op=ALU.add)
                rinv = mid.tile([8, 1], F32, tag="rinv", name="rinv", bufs=2)
                nc.vector.reciprocal(rinv[:], sm[:])
                ab = mid.tile([8, S], F16, tag="ab", name="ab", bufs=2)
                nc.vector.tensor_scalar_mul(ab[:], pb[:], rinv[:])
                ar = mid.tile([1, 512], F16, tag="ar", name="ar", bufs=2)
                nc.gpsimd.dma_start(
                    ar[0:1, :].rearrange("p (b s) -> p b s", b=8), ab[:])
                af = mid.tile([128, 512], F16, tag="af", name="af", bufs=2)
                nc.gpsimd.partition_broadcast(af[:], ar[:])
                # --- ct^T columns for these 8 batches, per h-block k ---
                for k in range(8):
                    pr = thp.tile([128, 512], F16, tag="prod", name="pr", bufs=3)
                    nc.vector.tensor_tensor(
                        pr[:], et[:, 512 * k:512 * (k + 1)], af[:], ALU.mult)
                    nc.vector.tensor_reduce(
                        ctT_sb[:, 64 * k + 8 * n:64 * k + 8 * n + 8],
                        pr[:].rearrange("p (b s) -> p b s", b=8),
                        axis=AX.X, op=ALU.add)
            # ct^T blocked [h-blk][b] is exactly xT blocks 2..9
            nc.vector.tensor_copy(xT_sb[:, 2 * BC:10 * BC], ctT_sb[:])

            I128_sb = cpool.tile([128, 128], F16, tag="I128")
            nc.sync.dma_start(I128_sb[:], d_I128[:])
            c0_sb = cpool.tile([BC, H], F32, tag="c0")
            nc.sync.dma_start(c0_sb[:], d_c0[:])
            c1_sb = cpool.tile([BC, H], F32, tag="c1")
            nc.sync.dma_start(c1_sb[:], d_c1[:])
            owT_sb = [cpool.tile([128, V], F16, tag=f"ow{k}", name=f"ow{k}") for k in range(8)]
            for k in range(8):
                nc.sync.dma_start(owT_sb[k][:], d_owT[128 * k:128 * (k + 1), :])
            ob_sb = cpool.tile([1, V], F16, tag="ob")
            nc.sync.dma_start(ob_sb[:], d_ob[:])

            # ---------- phases G/H: two LSTM layers ----------
            def lstm_layer(xT, n_xk, wxd, whd, hT, bias_d, c_in,
                           d_hout, d_cout, hTout, lname, pre_wx, pre_wh):
                """xT: SBUF [128, n_xk*64] input^T blocks; whd/wxd DRAM weights;
                hT: SBUF [128, 8*64] prev-h^T blocks; returns nothing."""
                gates = mid.tile([BC, G4], F16, tag="gates", name="gates")
                bt = mid.tile([1, G4], F16, tag="btile", name="bt")
                nc.sync.dma_start(bt[:], bias_d[:])
                for n in range(8):
                    wx = pre_wx.pop(n) if n in pre_wx else \
                        load_w(wxd, n_xk, n, nc.scalar)
                    wh = pre_wh.pop(n) if n in pre_wh else \
                        load_w(whd, 8, n, nc.sync)
                    ps = psS.tile([BC, 512], F32, tag="ps64", name="gps")
                    for k in range(n_xk):
                        nc.tensor.matmul(ps[:], xT[:, 64 * k:64 * (k + 1)],
                                         wx[:, 512 * k:512 * (k + 1)],
                                         start=(k == 0), stop=False)
                    for k in range(8):
                        nc.tensor.matmul(ps[:], hT[:, 64 * k:64 * (k + 1)],
                                         wh[:, 512 * k:512 * (k + 1)],
                                         start=False, stop=False)
                    nc.tensor.matmul(ps[:], ones1_sb[:],
                                     bt[:, 512 * n:512 * (n + 1)],
                                     start=False, stop=True)
                    func = AF.Tanh if n in (4, 5) else AF.Sigmoid
                    nc.scalar.activation(gates[:, 512 * n:512 * (n + 1)],
                                         ps[:], func)
                # c2 = sig_f*c + sig_i*tanh_g ; h2 = sig_o*tanh(c2)
                tmp = mid.tile([BC, H], F32, tag="lstm_tmp", name="tmp")
                nc.vector.tensor_tensor(tmp[:], gates[:, 0:H],
                                        gates[:, 2 * H:3 * H], ALU.mult)
                c2 = mid.tile([BC, H], F32, tag="c2t", name="c2")
                nc.vector.tensor_tensor(c2[:], gates[:, H:2 * H], c_in[:],
                                        ALU.mult)
                nc.vector.tensor_tensor(c2[:], c2[:], tmp[:], ALU.add)
                nc.sync.dma_start(d_cout[:], c2[:])
                tc2 = mid.tile([BC, H], F32, tag="lstm_tmp", name="tc2")
                nc.scalar.activation(tc2[:], c2[:], AF.Tanh)
                h2 = mid.tile([BC, H], F32, tag="h2t", name="h2")
                nc.vector.tensor_tensor(h2[:], gates[:, 3 * H:4 * H], tc2[:],
                                        ALU.mult)
                nc.sync.dma_start(d_hout[:], h2[:])
                h2f = mid.tile([BC, H], F16, tag="lstm_h2f", name="h2f")
                nc.vector.tensor_copy(h2f[:], h2[:])
                for j in range(8):
                    pt = psT.tile([128, BC], F16, tag="pT", name="trps")
                    nc.tensor.transpose(pt[:], h2f[:, 128 * j:128 * (j + 1)],
                                        I128_sb[0:64, 0:64])
                    nc.vector.tensor_copy(hTout[:, BC * j:BC * (j + 1)], pt[:])

            h0T_sb = mid.tile([128, 8 * BC], F16, tag="h0T")
            lstm_layer(xT_sb, 10, d_wi0, d_wh0, hT0_sb, d_b0, c0_sb,
                       d_h0n, d_c0n, h0T_sb, "l0", pre_wx0, pre_wh0)
            h1T_sb = mid.tile([128, 8 * BC], F16, tag="h1T")
            lstm_layer(h0T_sb, 8, d_wi1, d_wh1, hT1_sb, d_b1, c1_sb,
                       d_h1n, d_c1n, h1T_sb, "l1", {}, {})

            # ---------- phase I: logits ----------
            pl = psS.tile([BC, V], F32, tag="ps64", name="lps")
            for k in range(8):
                nc.tensor.matmul(pl[:], h1T_sb[:, 64 * k:64 * (k + 1)],
                                 owT_sb[k][:], start=(k == 0), stop=False)
            nc.tensor.matmul(pl[:], ones1_sb[:], ob_sb[:],
                             start=False, stop=True)
            lo = mid.tile([BC, V], F32, tag="lo")
            nc.vector.tensor_copy(lo[:], pl[:])
            nc.sync.dma_start(d_logits[:], lo[:])

    nc.compile()
    return nc


def _prep_inputs(input_ids, hidden, cell, encoder_outputs, emb, U, W, Vw,
                 Wih0, Whh0, bih0, bhh0, Wih1, Whh1, bih1, bhh1,
                 out_w, out_b):
    f16 = np.float16
    # shared across cores
    U16 = np.ascontiguousarray(U.astype(f16))
    W16 = np.ascontiguousarray(W.astype(f16))
    VwR = np.ascontiguousarray(Vw.reshape(8, 128).T.astype(f16))  # [128,8]
    emb16 = np.ascontiguousarray(emb.astype(f16))
    oh64 = np.zeros((BC, TOK), f16)
    for b in range(BC):
        oh64[b, 64 * b:64 * (b + 1)] = 1.0
    I128 = np.eye(128, dtype=f16)
    ones1 = np.ones((1, BC), f16)
    Wih0T = np.ascontiguousarray(Wih0.T.astype(f16))
    Whh0T = np.ascontiguousarray(Whh0.T.astype(f16))
    Wih1T = np.ascontiguousarray(Wih1.T.astype(f16))
    Whh1T = np.ascontiguousarray(Whh1.T.astype(f16))
    b0 = np.ascontiguousarray((bih0 + bhh0)[None, :].astype(f16))
    b1 = np.ascontiguousarray((bih1 + bhh1)[None, :].astype(f16))
    owT = np.ascontiguousarray(out_w.T.astype(f16))
    ob = np.ascontiguousarray(out_b[None, :].astype(f16))

    def blocked_T(x):  # [BC,H] -> [128, 8*BC] (k-blocks of columns)
        t = np.ascontiguousarray(x.T)          # [H, BC]
        return np.ascontiguousarray(
            t.reshape(8, 128, BC).transpose(1, 0, 2).reshape(128, 8 * BC)
        ).astype(f16)

    ids = np.asarray(input_ids).reshape(B)
    in_maps = []
    for c in range(NCORES):
        bs = slice(BC * c, BC * (c + 1))
        enc_c = encoder_outputs[bs]                      # [BC, S, H]
        encT = np.ascontiguousarray(
            enc_c.reshape(TOK, H).T.astype(f16))         # [H, TOK] b-major
        ohT = np.zeros((V, BC), f16)
        ohT[ids[bs].astype(np.int64), np.arange(BC)] = 1.0
        in_maps.append({
            "encT": encT, "Umat": U16, "Wmat": W16,
            "VwR": VwR,
            "hT0": blocked_T(hidden[0][bs]),
            "hT1": blocked_T(hidden[1][bs]),
            "onehotT": ohT, "embW": emb16, "oh64": oh64,
            "I128": I128, "ones1": ones1,
            "Wih0T": Wih0T, "Whh0T": Whh0T, "Wih1T": Wih1T, "Whh1T": Whh1T,
            "bias0": b0, "bias1": b1,
            "cell0": np.ascontiguousarray(cell[0][bs], dtype=np.float32),
            "cell1": np.ascontiguousarray(cell[1][bs], dtype=np.float32),
            "outWT": owT, "outB": ob,
        })
    return in_maps


def kernel(input_ids, hidden, cell, encoder_outputs, emb, U, W, Vw,
           Wih0, Whh0, bih0, bhh0, Wih1, Whh1, bih1, bhh1,
           out_w, out_b, matrix=0, _trace=False):
    if _COMPILED[0] is None:
        _COMPILED[0] = _build()
    nc = _COMPILED[0]
    args = [np.asarray(a) for a in
            (input_ids, hidden, cell, encoder_outputs, emb, U, W, Vw,
             Wih0, Whh0, bih0, bhh0, Wih1, Whh1, bih1, bhh1, out_w, out_b)]
    in_maps = _prep_inputs(*args)
    res = run_bass_kernel_spmd(nc, in_maps, core_ids=list(range(NCORES)),
                               trace=_trace)
    outs = res.results
    logits = np.concatenate([outs[c]["logits"] for c in range(NCORES)], 0)
    h_new = np.stack([
        np.concatenate([outs[c]["h0n"] for c in range(NCORES)], 0),
        np.concatenate([outs[c]["h1n"] for c in range(NCORES)], 0)])
    c_new = np.stack([
        np.concatenate([outs[c]["c0n"] for c in range(NCORES)], 0),
        np.concatenate([outs[c]["c1n"] for c in range(NCORES)], 0)])
    out = logits[:, None, :].astype(np.float32)
    kernel._last_results = res
    if int(np.asarray(matrix)):
        raise NotImplementedError("matrix=1 path not needed (reference uses 0)")
    return (out, h_new.astype(np.float32), c_new.astype(np.float32))


# revision 25
# speedup vs baseline: 1.1633x; 1.0078x over previous
"""Trainium2 Bass kernel for one attention-LSTM decoder step.

dims: B=512, S=64, H=1024, E=256, V=128, L=2, sharded data-parallel over
batch across 8 NeuronCores (64 batches/core). All matmuls run in fp16 with
fp32 PSUM accumulation; elementwise/softmax math in fp32.
"""

import sys

if "/opt/trn_rl_repo" not in sys.path:
    sys.path.insert(0, "/opt/trn_rl_repo")

import numpy as np

import concourse.bacc as bacc
import concourse.mybir as mybir
import concourse.tile as tile
from concourse.bass_utils import run_bass_kernel_spmd

B, S, H, E, V = 512, 64, 1024, 256, 128
NCORES = 8
BC = B // NCORES          # 64 batches per core
TOK = BC * S              # 4096 tokens per core
F = E + H                 # 1280 LSTM input features
G4 = 4 * H                # 4096 gate rows
F16 = mybir.dt.float16
F32 = mybir.dt.float32
AF = mybir.ActivationFunctionType
ALU = mybir.AluOpType
AX = mybir.AxisListType

_COMPILED = [None]


def _build():
    nc = bacc.Bacc("TRN2", target_bir_lowering=False, debug=False,
                   num_devices=NCORES)

    # ---- DRAM I/O ----
    d_encT = nc.dram_tensor("encT", [H, TOK], F16, kind="ExternalInput")
    d_U = nc.dram_tensor("Umat", [H, H], F16, kind="ExternalInput")
    d_W = nc.dram_tensor("Wmat", [H, H], F16, kind="ExternalInput")
    d_Vw = nc.dram_tensor("VwR", [128, 8], F16, kind="ExternalInput")
    d_hT0 = nc.dram_tensor("hT0", [128, 8 * BC], F16, kind="ExternalInput")
    d_hT1 = nc.dram_tensor("hT1", [128, 8 * BC], F16, kind="ExternalInput")
    d_ohT = nc.dram_tensor("onehotT", [V, BC], F16, kind="ExternalInput")
    d_emb = nc.dram_tensor("embW", [V, E], F16, kind="ExternalInput")
    d_oh64 = nc.dram_tensor("oh64", [BC, TOK], F16, kind="ExternalInput")
    d_I128 = nc.dram_tensor("I128", [128, 128], F16, kind="ExternalInput")
    d_ones1 = nc.dram_tensor("ones1", [1, BC], F16, kind="ExternalInput")
    d_wi0 = nc.dram_tensor("Wih0T", [F, G4], F16, kind="ExternalInput")
    d_wh0 = nc.dram_tensor("Whh0T", [H, G4], F16, kind="ExternalInput")
    d_wi1 = nc.dram_tensor("Wih1T", [H, G4], F16, kind="ExternalInput")
    d_wh1 = nc.dram_tensor("Whh1T", [H, G4], F16, kind="ExternalInput")
    d_b0 = nc.dram_tensor("bias0", [1, G4], F16, kind="ExternalInput")
    d_b1 = nc.dram_tensor("bias1", [1, G4], F16, kind="ExternalInput")
    d_c0 = nc.dram_tensor("cell0", [BC, H], F32, kind="ExternalInput")
    d_c1 = nc.dram_tensor("cell1", [BC, H], F32, kind="ExternalInput")
    d_owT = nc.dram_tensor("outWT", [H, V], F16, kind="ExternalInput")
    d_ob = nc.dram_tensor("outB", [1, V], F16, kind="ExternalInput")

    d_logits = nc.dram_tensor("logits", [BC, V], F32, kind="ExternalOutput")
    d_h0n = nc.dram_tensor("h0n", [BC, H], F32, kind="ExternalOutput")
    d_h1n = nc.dram_tensor("h1n", [BC, H], F32, kind="ExternalOutput")
    d_c0n = nc.dram_tensor("c0n", [BC, H], F32, kind="ExternalOutput")
    d_c1n = nc.dram_tensor("c1n", [BC, H], F32, kind="ExternalOutput")

    with tile.TileContext(nc) as tc:
        with (
            tc.tile_pool(name="const", bufs=1) as cpool,
            tc.tile_pool(name="stream", bufs=3) as stp,
            tc.tile_pool(name="tanh", bufs=3) as thp,
            tc.tile_pool(name="wls", bufs=3) as wpool,
            tc.tile_pool(name="mid", bufs=1) as mid,
            tc.tile_pool(name="psA", bufs=3, space="PSUM") as psA,
            tc.tile_pool(name="psE", bufs=1, space="PSUM") as psE,
            tc.tile_pool(name="psS", bufs=2, space="PSUM") as psS,
            tc.tile_pool(name="psT", bufs=2, space="PSUM") as psT,
        ):
            # ---------- resident constants ----------
            vw_sb = cpool.tile([128, 8], F16, tag="vw")
            nc.sync.dma_start(vw_sb[:], d_Vw[:])
            hT0_sb = cpool.tile([128, 8 * BC], F16, tag="hT0")
            nc.sync.dma_start(hT0_sb[:], d_hT0[:])
            hT1_sb = cpool.tile([128, 8 * BC], F16, tag="hT1")
            nc.sync.dma_start(hT1_sb[:], d_hT1[:])
            ohT_sb = cpool.tile([V, BC], F16, tag="ohT")
            nc.sync.dma_start(ohT_sb[:], d_ohT[:])
            emb_sb = cpool.tile([V, E], F16, tag="emb")
            nc.sync.dma_start(emb_sb[:], d_emb[:])
            ones1_sb = cpool.tile([1, BC], F16, tag="ones1")
            nc.sync.dma_start(ones1_sb[:], d_ones1[:])

            # ---------- phase A: t2 = h_top @ W  -> [BC, H] f16 ----------
            t2_sb = mid.tile([BC, H], F16, tag="t2")
            for n2 in range(2):
                ps = psS.tile([BC, 512], F32, tag="ps64", name="t2ps")
                wsl = stp.tile([128, 4096], F16, tag="big", name="wsl")
                nc.sync.dma_start(
                    wsl[:].rearrange("p (k c) -> p k c", c=512),
                    d_W.rearrange("(k p) h -> k p h", p=128)
                       [:, :, 512 * n2:512 * (n2 + 1)]
                       .transpose([1, 0, 2]))
                for k in range(8):
                    nc.tensor.matmul(
                        ps[:], hT1_sb[:, 64 * k:64 * (k + 1)],
                        wsl[:, 512 * k:512 * (k + 1)],
                        start=(k == 0), stop=(k == 7))
                nc.vector.tensor_copy(t2_sb[:, 512 * n2:512 * (n2 + 1)], ps[:])

            U_sb = [cpool.tile([128, H], F16, tag=f"U{k}", name=f"U{k}") for k in range(8)]
            for k in range(8):
                nc.scalar.dma_start(U_sb[k][:], d_U[128 * k:128 * (k + 1), :])
            # ---------- phase B: embedded^T -> xT blocks 0..1 ----------
            xT_sb = mid.tile([128, 10 * BC], F16, tag="xT")
            for et in range(2):
                ps = psT.tile([128, BC], F32, tag="pT", name="embps")
                nc.tensor.matmul(ps[:], emb_sb[:, 128 * et:128 * (et + 1)],
                                 ohT_sb[:], start=True, stop=True)
                nc.vector.tensor_copy(xT_sb[:, BC * et:BC * (et + 1)], ps[:])

            def load_et(n):
                t = stp.tile([128, 4096], F16, tag="big", name=f"et{n}")
                nc.sync.dma_start(
                    t[:].rearrange("p (k c) -> p k c", c=512),
                    d_encT.rearrange("(k p) t -> k p t", p=128)
                          [:, :, 512 * n:512 * (n + 1)]
                          .transpose([1, 0, 2]))
                return t

            def load_oh(n):
                t = stp.tile([BC, 512], F16, tag="oh", name=f"oh{n}")
                nc.sync.dma_start(t[:], d_oh64[:, 512 * n:512 * (n + 1)])
                return t

            pre_et = {n: load_et(n) for n in range(2)}
            pre_oh = {n: load_oh(n) for n in range(2)}
            def load_w(wd, n_k, n, eng):
                t = wpool.tile([128, n_k * 512], F16,
                               tag="wx" if n_k > 8 else "wh", name=f"w{n}")
                eng.dma_start(
                    t[:].rearrange("p (k c) -> p k c", c=512),
                    wd.rearrange("(k p) g -> k p g", p=128)
                      [:, :, 512 * n:512 * (n + 1)].transpose([1, 0, 2]))
                return t

            pre_wx0 = {n: load_w(d_wi0, 10, n, nc.scalar) for n in range(3)}
            pre_wh0 = {n: load_w(d_wh0, 8, n, nc.sync) for n in range(3)}

            # ---------- phase C: t1 + t2 -> tanh -> scores -> softmax -> ct ---
            # token tile n holds ALL 64 s-positions of batches 8n..8n+8, so
            # each tile's softmax + attention-context can be computed inline
            # against the already-resident encT tile (no second enc stream).
            ctT_sb = mid.tile([128, 512], F32, tag="ctT")  # [h-blk k][8n+b] cols
            for n in range(8):          # token tiles (512 tokens, 8 batches)
                et = pre_et.pop(n) if n in pre_et else load_et(n)
                oh64_t = pre_oh.pop(n) if n in pre_oh else load_oh(n)
                pe = psE.tile([1, 512], F32, tag="eps")
                for m in range(8):      # output-H tiles
                    pt = psA.tile([128, 512], F32, tag="t1ps")
                    for k in range(8):
                        nc.tensor.matmul(
                            pt[:], U_sb[k][:, 128 * m:128 * (m + 1)],
                            et[:, 512 * k:512 * (k + 1)],
                            start=(k == 0), stop=False)
                    # inject t2 broadcast over s:  lhsT=[64b,128h'] rhs=[64b,512tok]
                    nc.tensor.matmul(
                        pt[:], t2_sb[:, 128 * m:128 * (m + 1)],
                        oh64_t[:], start=False, stop=True)
                    th = thp.tile([128, 512], F16, tag="tanh")
                    nc.scalar.activation(th[:], pt[:], AF.Tanh)
                    nc.tensor.matmul(pe[:], vw_sb[:, m:m + 1], th[:],
                                     start=(m == 0), stop=(m == 7))
                # --- inline softmax over s for batches 8n..8n+8 ---
                er = mid.tile([1, 512], F32, tag="er", name="er", bufs=2)
                nc.vector.tensor_copy(er[:], pe[:])
                eb = mid.tile([8, S], F32, tag="eb", name="eb", bufs=2)
                nc.gpsimd.dma_start(
                    eb[:], er[0:1, :].rearrange("p (b s) -> p b s", b=8))
                mx = mid.tile([8, 1], F32, tag="mx", name="mx", bufs=2)
                nc.vector.tensor_reduce(mx[:], eb[:], axis=AX.X, op=ALU.max)
                negmx = mid.tile([8, 1], F32, tag="negmx", name="negmx", bufs=2)
                nc.vector.tensor_scalar_mul(negmx[:], mx[:], -1.0)
                pb = mid.tile([8, S], F32, tag="pb", name="pb", bufs=2)
                nc.scalar.activation(pb[:], eb[:], AF.Exp, bias=negmx[:])
                sm = mid.tile([8, 1], F32, tag="sm", name="sm", bufs=2)
                nc.vector.tensor_reduce(sm[:], pb[:], axis=AX.X, op=ALU.add)
                rinv = mid.tile([8, 1], F32, tag="rinv", name="rinv", bufs=2)
                nc.vector.reciprocal(rinv[:], sm[:])
                ab = mid.tile([8, S], F16, tag="ab", name="ab", bufs=2)
                nc.vector.tensor_scalar_mul(ab[:], pb[:], rinv[:])
                ar = mid.tile([1, 512], F16, tag="ar", name="ar", bufs=2)
                nc.gpsimd.dma_start(
                    ar[0:1, :].rearrange("p (b s) -> p b s", b=8), ab[:])
                af = mid.tile([128, 512], F16, tag="af", name="af", bufs=2)
                nc.gpsimd.partition_broadcast(af[:], ar[:])
                # --- ct^T columns for these 8 batches, per h-block k ---
                for k in range(8):
                    pr = thp.tile([128, 512], F16, tag="prod", name="pr", bufs=3)
                    nc.vector.tensor_tensor(
                        pr[:], et[:, 512 * k:512 * (k + 1)], af[:], ALU.mult)
                    nc.vector.tensor_reduce(
                        ctT_sb[:, 64 * k + 8 * n:64 * k + 8 * n + 8],
                        pr[:].rearrange("p (b s) -> p b s", b=8),
                        axis=AX.X, op=ALU.add)
                    if n == 7:
                        # block k of ct^T is now complete -> xT block 2+k
                        nc.vector.tensor_copy(
                            xT_sb[:, BC * (2 + k):BC * (3 + k)],
                            ctT_sb[:, 64 * k:64 * (k + 1)])

            I128_sb = cpool.tile([128, 128], F16, tag="I128")
            nc.sync.dma_start(I128_sb[:], d_I128[:])
            c0_sb = cpool.tile([BC, H], F32, tag="c0")
            nc.sync.dma_start(c0_sb[:], d_c0[:])
            c1_sb = cpool.tile([BC, H], F32, tag="c1")
            nc.sync.dma_start(c1_sb[:], d_c1[:])
            owT_sb = [cpool.tile([128, V], F16, tag=f"ow{k}", name=f"ow{k}") for k in range(8)]
            for k in range(8):
                nc.sync.dma_start(owT_sb[k][:], d_owT[128 * k:128 * (k + 1), :])
            ob_sb = cpool.tile([1, V], F16, tag="ob")
            nc.sync.dma_start(ob_sb[:], d_ob[:])

            # ---------- phases G/H: two LSTM layers ----------
            def lstm_layer(xT, n_xk, wxd, whd, hT, bias_d, c_in,
                           d_hout, d_cout, hTout, lname, pre_wx, pre_wh):
                """xT: SBUF [128, n_xk*64] input^T blocks; whd/wxd DRAM weights;
                hT: SBUF [128, 8*64] prev-h^T blocks; returns nothing."""
                gates = mid.tile([BC, G4], F16, tag="gates", name="gates")
                bt = mid.tile([1, G4], F16, tag="btile", name="bt")
                nc.sync.dma_start(bt[:], bias_d[:])
                for n in range(8):
                    wx = pre_wx.pop(n) if n in pre_wx else \
                        load_w(wxd, n_xk, n, nc.scalar)
                    wh = pre_wh.pop(n) if n in pre_wh else \
                        load_w(whd, 8, n, nc.sync)
                    ps = psS.tile([BC, 512], F32, tag="ps64", name="gps")
                    for k in range(n_xk):
                        nc.tensor.matmul(ps[:], xT[:, 64 * k:64 * (k + 1)],
                                         wx[:, 512 * k:512 * (k + 1)],
                                         start=(k == 0), stop=False)
                    for k in range(8):
                        nc.tensor.matmul(ps[:], hT[:, 64 * k:64 * (k + 1)],
                                         wh[:, 512 * k:512 * (k + 1)],
                                         start=False, stop=False)
                    nc.tensor.matmul(ps[:], ones1_sb[:],
                                     bt[:, 512 * n:512 * (n + 1)],
                                     start=False, stop=True)
                    func = AF.Tanh if n in (4, 5) else AF.Sigmoid
                    nc.scalar.activation(gates[:, 512 * n:512 * (n + 1)],
                                         ps[:], func)
                # c2 = sig_f*c + sig_i*tanh_g ; h2 = sig_o*tanh(c2)
                tmp = mid.tile([BC, H], F32, tag="lstm_tmp", name="tmp")
                nc.vector.tensor_tensor(tmp[:], gates[:, 0:H],
                                        gates[:, 2 * H:3 * H], ALU.mult)
                c2 = mid.tile([BC, H], F32, tag="c2t", name="c2")
                nc.vector.tensor_tensor(c2[:], gates[:, H:2 * H], c_in[:],
                                        ALU.mult)
                nc.vector.tensor_tensor(c2[:], c2[:], tmp[:], ALU.add)
                nc.sync.dma_start(d_cout[:], c2[:])
                tc2 = mid.tile([BC, H], F32, tag="lstm_tmp", name="tc2")
                nc.scalar.activation(tc2[:], c2[:], AF.Tanh)
                h2 = mid.tile([BC, H], F32, tag="h2t", name="h2")
                nc.vector.tensor_tensor(h2[:], gates[:, 3 * H:4 * H], tc2[:],
                                        ALU.mult)
                nc.sync.dma_start(d_hout[:], h2[:])
                h2f = mid.tile([BC, H], F16, tag="lstm_h2f", name="h2f")
                nc.vector.tensor_copy(h2f[:], h2[:])
                for j in range(8):
                    pt = psT.tile([128, BC], F16, tag="pT", name="trps")
                    nc.tensor.transpose(pt[:], h2f[:, 128 * j:128 * (j + 1)],
                                        I128_sb[0:64, 0:64])
                    nc.vector.tensor_copy(hTout[:, BC * j:BC * (j + 1)], pt[:])

            h0T_sb = mid.tile([128, 8 * BC], F16, tag="h0T")
            lstm_layer(xT_sb, 10, d_wi0, d_wh0, hT0_sb, d_b0, c0_sb,
                       d_h0n, d_c0n, h0T_sb, "l0", pre_wx0, pre_wh0)
            h1T_sb = mid.tile([128, 8 * BC], F16, tag="h1T")
            lstm_layer(h0T_sb, 8, d_wi1, d_wh1, hT1_sb, d_b1, c1_sb,
                       d_h1n, d_c1n, h1T_sb, "l1", {}, {})

            # ---------- phase I: logits ----------
            pl = psS.tile([BC, V], F32, tag="ps64", name="lps")
            for k in range(8):
                nc.tensor.matmul(pl[:], h1T_sb[:, 64 * k:64 * (k + 1)],
                                 owT_sb[k][:], start=(k == 0), stop=False)
            nc.tensor.matmul(pl[:], ones1_sb[:], ob_sb[:],
                             start=False, stop=True)
            lo = mid.tile([BC, V], F32, tag="lo")
            nc.vector.tensor_copy(lo[:], pl[:])
            nc.sync.dma_start(d_logits[:], lo[:])

    nc.compile()
    return nc


def _prep_inputs(input_ids, hidden, cell, encoder_outputs, emb, U, W, Vw,
                 Wih0, Whh0, bih0, bhh0, Wih1, Whh1, bih1, bhh1,
                 out_w, out_b):
    f16 = np.float16
    # shared across cores
    U16 = np.ascontiguousarray(U.astype(f16))
    W16 = np.ascontiguousarray(W.astype(f16))
    VwR = np.ascontiguousarray(Vw.reshape(8, 128).T.astype(f16))  # [128,8]
    emb16 = np.ascontiguousarray(emb.astype(f16))
    oh64 = np.zeros((BC, TOK), f16)
    for b in range(BC):
        oh64[b, 64 * b:64 * (b + 1)] = 1.0
    I128 = np.eye(128, dtype=f16)
    ones1 = np.ones((1, BC), f16)
    Wih0T = np.ascontiguousarray(Wih0.T.astype(f16))
    Whh0T = np.ascontiguousarray(Whh0.T.astype(f16))
    Wih1T = np.ascontiguousarray(Wih1.T.astype(f16))
    Whh1T = np.ascontiguousarray(Whh1.T.astype(f16))
    b0 = np.ascontiguousarray((bih0 + bhh0)[None, :].astype(f16))
    b1 = np.ascontiguousarray((bih1 + bhh1)[None, :].astype(f16))
    owT = np.ascontiguousarray(out_w.T.astype(f16))
    ob = np.ascontiguousarray(out_b[None, :].astype(f16))

    def blocked_T(x):  # [BC,H] -> [128, 8*BC] (k-blocks of columns)
        t = np.ascontiguousarray(x.T)          # [H, BC]
        return np.ascontiguousarray(
            t.reshape(8, 128, BC).transpose(1, 0, 2).reshape(128, 8 * BC)
        ).astype(f16)

    ids = np.asarray(input_ids).reshape(B)
    in_maps = []
    for c in range(NCORES):
        bs = slice(BC * c, BC * (c + 1))
        enc_c = encoder_outputs[bs]                      # [BC, S, H]
        encT = np.ascontiguousarray(
            enc_c.reshape(TOK, H).T.astype(f16))         # [H, TOK] b-major
        ohT = np.zeros((V, BC), f16)
        ohT[ids[bs].astype(np.int64), np.arange(BC)] = 1.0
        in_maps.append({
            "encT": encT, "Umat": U16, "Wmat": W16,
            "VwR": VwR,
            "hT0": blocked_T(hidden[0][bs]),
            "hT1": blocked_T(hidden[1][bs]),
            "onehotT": ohT, "embW": emb16, "oh64": oh64,
            "I128": I128, "ones1": ones1,
            "Wih0T": Wih0T, "Whh0T": Whh0T, "Wih1T": Wih1T, "Whh1T": Whh1T,
            "bias0": b0, "bias1": b1,
            "cell0": np.ascontiguousarray(cell[0][bs], dtype=np.float32),
            "cell1": np.ascontiguousarray(cell[1][bs], dtype=np.float32),
            "outWT": owT, "outB": ob,
        })
    return in_maps


def kernel(input_ids, hidden, cell, encoder_outputs, emb, U, W, Vw,
           Wih0, Whh0, bih0, bhh0, Wih1, Whh1, bih1, bhh1,
           out_w, out_b, matrix=0, _trace=False):
    if _COMPILED[0] is None:
        _COMPILED[0] = _build()
    nc = _COMPILED[0]
    args = [np.asarray(a) for a in
            (input_ids, hidden, cell, encoder_outputs, emb, U, W, Vw,
             Wih0, Whh0, bih0, bhh0, Wih1, Whh1, bih1, bhh1, out_w, out_b)]
    in_maps = _prep_inputs(*args)
    res = run_bass_kernel_spmd(nc, in_maps, core_ids=list(range(NCORES)),
                               trace=_trace)
    outs = res.results
    logits = np.concatenate([outs[c]["logits"] for c in range(NCORES)], 0)
    h_new = np.stack([
        np.concatenate([outs[c]["h0n"] for c in range(NCORES)], 0),
        np.concatenate([outs[c]["h1n"] for c in range(NCORES)], 0)])
    c_new = np.stack([
        np.concatenate([outs[c]["c0n"] for c in range(NCORES)], 0),
        np.concatenate([outs[c]["c1n"] for c in range(NCORES)], 0)])
    out = logits[:, None, :].astype(np.float32)
    kernel._last_results = res
    if int(np.asarray(matrix)):
        raise NotImplementedError("matrix=1 path not needed (reference uses 0)")
    return (out, h_new.astype(np.float32), c_new.astype(np.float32))


# revision 26
# speedup vs baseline: 1.1965x; 1.0285x over previous
"""Trainium2 Bass kernel for one attention-LSTM decoder step.

dims: B=512, S=64, H=1024, E=256, V=128, L=2, sharded data-parallel over
batch across 8 NeuronCores (64 batches/core). All matmuls run in fp16 with
fp32 PSUM accumulation; elementwise/softmax math in fp32.
"""

import sys

if "/opt/trn_rl_repo" not in sys.path:
    sys.path.insert(0, "/opt/trn_rl_repo")

import numpy as np

import concourse.bacc as bacc
import concourse.mybir as mybir
import concourse.tile as tile
from concourse.bass_utils import run_bass_kernel_spmd

B, S, H, E, V = 512, 64, 1024, 256, 128
NCORES = 8
BC = B // NCORES          # 64 batches per core
TOK = BC * S              # 4096 tokens per core
F = E + H                 # 1280 LSTM input features
G4 = 4 * H                # 4096 gate rows
F16 = mybir.dt.float16
F32 = mybir.dt.float32
AF = mybir.ActivationFunctionType
ALU = mybir.AluOpType
AX = mybir.AxisListType

_COMPILED = [None]


def _build():
    nc = bacc.Bacc("TRN2", target_bir_lowering=False, debug=False,
                   num_devices=NCORES)

    # ---- DRAM I/O ----
    d_encT = nc.dram_tensor("encT", [H, TOK], F16, kind="ExternalInput")
    d_U = nc.dram_tensor("Umat", [H, H], F16, kind="ExternalInput")
    d_W = nc.dram_tensor("Wmat", [H, H], F16, kind="ExternalInput")
    d_Vw = nc.dram_tensor("VwR", [128, 8], F16, kind="ExternalInput")
    d_hT0 = nc.dram_tensor("hT0", [128, 8 * BC], F16, kind="ExternalInput")
    d_hT1 = nc.dram_tensor("hT1", [128, 8 * BC], F16, kind="ExternalInput")
    d_ohT = nc.dram_tensor("onehotT", [V, BC], F16, kind="ExternalInput")
    d_emb = nc.dram_tensor("embW", [V, E], F16, kind="ExternalInput")
    d_oh64 = nc.dram_tensor("oh64", [BC, TOK], F16, kind="ExternalInput")
    d_I128 = nc.dram_tensor("I128", [128, 128], F16, kind="ExternalInput")
    d_ones1 = nc.dram_tensor("ones1", [1, BC], F16, kind="ExternalInput")
    d_wi0 = nc.dram_tensor("Wih0T", [F, G4], F16, kind="ExternalInput")
    d_wh0 = nc.dram_tensor("Whh0T", [H, G4], F16, kind="ExternalInput")
    d_wi1 = nc.dram_tensor("Wih1T", [H, G4], F16, kind="ExternalInput")
    d_wh1 = nc.dram_tensor("Whh1T", [H, G4], F16, kind="ExternalInput")
    d_b0 = nc.dram_tensor("bias0", [1, G4], F16, kind="ExternalInput")
    d_b1 = nc.dram_tensor("bias1", [1, G4], F16, kind="ExternalInput")
    d_c0 = nc.dram_tensor("cell0", [BC, H], F32, kind="ExternalInput")
    d_c1 = nc.dram_tensor("cell1", [BC, H], F32, kind="ExternalInput")
    d_owT = nc.dram_tensor("outWT", [H, V], F16, kind="ExternalInput")
    d_ob = nc.dram_tensor("outB", [1, V], F16, kind="ExternalInput")

    d_logits = nc.dram_tensor("logits", [BC, V], F32, kind="ExternalOutput")
    d_h0n = nc.dram_tensor("h0n", [BC, H], F32, kind="ExternalOutput")
    d_h1n = nc.dram_tensor("h1n", [BC, H], F32, kind="ExternalOutput")
    d_c0n = nc.dram_tensor("c0n", [BC, H], F32, kind="ExternalOutput")
    d_c1n = nc.dram_tensor("c1n", [BC, H], F32, kind="ExternalOutput")

    with tile.TileContext(nc) as tc:
        with (
            tc.tile_pool(name="const", bufs=1) as cpool,
            tc.tile_pool(name="stream", bufs=3) as stp,
            tc.tile_pool(name="tanh", bufs=3) as thp,
            tc.tile_pool(name="wls", bufs=3) as wpool,
            tc.tile_pool(name="mid", bufs=1) as mid,
            tc.tile_pool(name="psA", bufs=3, space="PSUM") as psA,
            tc.tile_pool(name="psE", bufs=1, space="PSUM") as psE,
            tc.tile_pool(name="psS", bufs=2, space="PSUM") as psS,
            tc.tile_pool(name="psT", bufs=2, space="PSUM") as psT,
        ):
            # ---------- resident constants ----------
            vw_sb = cpool.tile([128, 8], F16, tag="vw")
            nc.sync.dma_start(vw_sb[:], d_Vw[:])
            hT0_sb = cpool.tile([128, 8 * BC], F16, tag="hT0")
            nc.sync.dma_start(hT0_sb[:], d_hT0[:])
            hT1_sb = cpool.tile([128, 8 * BC], F16, tag="hT1")
            nc.sync.dma_start(hT1_sb[:], d_hT1[:])
            ohT_sb = cpool.tile([V, BC], F16, tag="ohT")
            nc.sync.dma_start(ohT_sb[:], d_ohT[:])
            emb_sb = cpool.tile([V, E], F16, tag="emb")
            nc.sync.dma_start(emb_sb[:], d_emb[:])
            ones1_sb = cpool.tile([1, BC], F16, tag="ones1")
            nc.sync.dma_start(ones1_sb[:], d_ones1[:])

            # ---------- phase A: t2 = h_top @ W  -> [BC, H] f16 ----------
            t2_sb = mid.tile([BC, H], F16, tag="t2")
            for n2 in range(2):
                ps = psS.tile([BC, 512], F32, tag="ps64", name="t2ps")
                wsl = stp.tile([128, 4096], F16, tag="big", name="wsl")
                nc.sync.dma_start(
                    wsl[:].rearrange("p (k c) -> p k c", c=512),
                    d_W.rearrange("(k p) h -> k p h", p=128)
                       [:, :, 512 * n2:512 * (n2 + 1)]
                       .transpose([1, 0, 2]))
                for k in range(8):
                    nc.tensor.matmul(
                        ps[:], hT1_sb[:, 64 * k:64 * (k + 1)],
                        wsl[:, 512 * k:512 * (k + 1)],
                        start=(k == 0), stop=(k == 7))
                nc.vector.tensor_copy(t2_sb[:, 512 * n2:512 * (n2 + 1)], ps[:])

            U_sb = [cpool.tile([128, H], F16, tag=f"U{k}", name=f"U{k}") for k in range(8)]
            for k in range(8):
                nc.scalar.dma_start(U_sb[k][:], d_U[128 * k:128 * (k + 1), :])
            # ---------- phase B: embedded^T -> xT blocks 0..1 ----------
            xT_sb = mid.tile([128, 10 * BC], F16, tag="xT")
            for et in range(2):
                ps = psT.tile([128, BC], F32, tag="pT", name="embps")
                nc.tensor.matmul(ps[:], emb_sb[:, 128 * et:128 * (et + 1)],
                                 ohT_sb[:], start=True, stop=True)
                nc.vector.tensor_copy(xT_sb[:, BC * et:BC * (et + 1)], ps[:])

            def load_et(n):
                t = stp.tile([128, 4096], F16, tag="big", name=f"et{n}")
                nc.sync.dma_start(
                    t[:].rearrange("p (k c) -> p k c", c=512),
                    d_encT.rearrange("(k p) t -> k p t", p=128)
                          [:, :, 512 * n:512 * (n + 1)]
                          .transpose([1, 0, 2]))
                return t

            def load_oh(n):
                t = stp.tile([BC, 512], F16, tag="oh", name=f"oh{n}")
                nc.sync.dma_start(t[:], d_oh64[:, 512 * n:512 * (n + 1)])
                return t

            pre_et = {n: load_et(n) for n in range(2)}
            pre_oh = {n: load_oh(n) for n in range(2)}
            def load_w(wd, n_k, n, eng):
                t = wpool.tile([128, n_k * 512], F16,
                               tag="wx" if n_k > 8 else "wh", name=f"w{n}")
                eng.dma_start(
                    t[:].rearrange("p (k c) -> p k c", c=512),
                    wd.rearrange("(k p) g -> k p g", p=128)
                      [:, :, 512 * n:512 * (n + 1)].transpose([1, 0, 2]))
                return t

            pre_wx0 = {n: load_w(d_wi0, 10, n, nc.scalar) for n in range(3)}
            pre_wh0 = {n: load_w(d_wh0, 8, n, nc.sync) for n in range(3)}

            # ---------- phase C: t1 + t2 -> tanh -> scores -> softmax -> ct ---
            # token tile n holds ALL 64 s-positions of batches 8n..8n+8, so
            # each tile's softmax + attention-context can be computed inline
            # against the already-resident encT tile (no second enc stream).
            ctT_sb = mid.tile([128, 512], F32, tag="ctT")  # [h-blk k][8n+b] cols
            for n in range(8):          # token tiles (512 tokens, 8 batches)
                et = pre_et.pop(n) if n in pre_et else load_et(n)
                oh64_t = pre_oh.pop(n) if n in pre_oh else load_oh(n)
                pe = psE.tile([1, 512], F32, tag="eps")
                ths = []
                for m in range(8):      # output-H tiles
                    pt = psA.tile([128, 512], F32, tag="t1ps")
                    for k in range(8):
                        nc.tensor.matmul(
                            pt[:], U_sb[k][:, 128 * m:128 * (m + 1)],
                            et[:, 512 * k:512 * (k + 1)],
                            start=(k == 0), stop=False)
                    # inject t2 broadcast over s:  lhsT=[64b,128h'] rhs=[64b,512tok]
                    nc.tensor.matmul(
                        pt[:], t2_sb[:, 128 * m:128 * (m + 1)],
                        oh64_t[:], start=False, stop=True)
                    th = thp.tile([128, 512], F16, tag="tanh", name=f"th{m}",
                                  bufs=9)
                    nc.scalar.activation(th[:], pt[:], AF.Tanh)
                    ths.append(th)
                # batched scores: Vw stays loaded across the 8 matmuls
                for m in range(8):
                    nc.tensor.matmul(pe[:], vw_sb[:, m:m + 1], ths[m][:],
                                     start=(m == 0), stop=(m == 7))
                # --- inline softmax over s for batches 8n..8n+8 ---
                er = mid.tile([1, 512], F32, tag="er", name="er", bufs=2)
                nc.vector.tensor_copy(er[:], pe[:])
                eb = mid.tile([8, S], F32, tag="eb", name="eb", bufs=2)
                nc.gpsimd.dma_start(
                    eb[:], er[0:1, :].rearrange("p (b s) -> p b s", b=8))
                mx = mid.tile([8, 1], F32, tag="mx", name="mx", bufs=2)
                nc.vector.tensor_reduce(mx[:], eb[:], axis=AX.X, op=ALU.max)
                negmx = mid.tile([8, 1], F32, tag="negmx", name="negmx", bufs=2)
                nc.vector.tensor_scalar_mul(negmx[:], mx[:], -1.0)
                pb = mid.tile([8, S], F32, tag="pb", name="pb", bufs=2)
                nc.scalar.activation(pb[:], eb[:], AF.Exp, bias=negmx[:])
                sm = mid.tile([8, 1], F32, tag="sm", name="sm", bufs=2)
                nc.vector.tensor_reduce(sm[:], pb[:], axis=AX.X, op=ALU.add)
                rinv = mid.tile([8, 1], F32, tag="rinv", name="rinv", bufs=2)
                nc.vector.reciprocal(rinv[:], sm[:])
                ab = mid.tile([8, S], F16, tag="ab", name="ab", bufs=2)
                nc.vector.tensor_scalar_mul(ab[:], pb[:], rinv[:])
                ar = mid.tile([1, 512], F16, tag="ar", name="ar", bufs=2)
                nc.gpsimd.dma_start(
                    ar[0:1, :].rearrange("p (b s) -> p b s", b=8), ab[:])
                af = mid.tile([128, 512], F16, tag="af", name="af", bufs=2)
                nc.gpsimd.partition_broadcast(af[:], ar[:])
                # --- ct^T columns for these 8 batches, per h-block k ---
                for k in range(8):
                    pr = thp.tile([128, 512], F16, tag="prod", name="pr", bufs=3)
                    nc.vector.tensor_tensor(
                        pr[:], et[:, 512 * k:512 * (k + 1)], af[:], ALU.mult)
                    nc.vector.tensor_reduce(
                        ctT_sb[:, 64 * k + 8 * n:64 * k + 8 * n + 8],
                        pr[:].rearrange("p (b s) -> p b s", b=8),
                        axis=AX.X, op=ALU.add)
                    if n == 7:
                        # block k of ct^T is now complete -> xT block 2+k
                        nc.vector.tensor_copy(
                            xT_sb[:, BC * (2 + k):BC * (3 + k)],
                            ctT_sb[:, 64 * k:64 * (k + 1)])

            I128_sb = cpool.tile([128, 128], F16, tag="I128")
            nc.sync.dma_start(I128_sb[:], d_I128[:])
            c0_sb = cpool.tile([BC, H], F32, tag="c0")
            nc.sync.dma_start(c0_sb[:], d_c0[:])
            c1_sb = cpool.tile([BC, H], F32, tag="c1")
            nc.sync.dma_start(c1_sb[:], d_c1[:])
            owT_sb = [cpool.tile([128, V], F16, tag=f"ow{k}", name=f"ow{k}") for k in range(8)]
            for k in range(8):
                nc.sync.dma_start(owT_sb[k][:], d_owT[128 * k:128 * (k + 1), :])
            ob_sb = cpool.tile([1, V], F16, tag="ob")
            nc.sync.dma_start(ob_sb[:], d_ob[:])

            # ---------- phases G/H: two LSTM layers ----------
            def lstm_layer(xT, n_xk, wxd, whd, hT, bias_d, c_in,
                           d_hout, d_cout, hTout, lname, pre_wx, pre_wh):
                """xT: SBUF [128, n_xk*64] input^T blocks; whd/wxd DRAM weights;
                hT: SBUF [128, 8*64] prev-h^T blocks; returns nothing."""
                gates = mid.tile([BC, G4], F16, tag="gates", name="gates")
                bt = mid.tile([1, G4], F16, tag="btile", name="bt")
                nc.sync.dma_start(bt[:], bias_d[:])
                for n in range(8):
                    wx = pre_wx.pop(n) if n in pre_wx else \
                        load_w(wxd, n_xk, n, nc.scalar)
                    wh = pre_wh.pop(n) if n in pre_wh else \
                        load_w(whd, 8, n, nc.sync)
                    ps = psS.tile([BC, 512], F32, tag="ps64", name="gps")
                    for k in range(n_xk):
                        nc.tensor.matmul(ps[:], xT[:, 64 * k:64 * (k + 1)],
                                         wx[:, 512 * k:512 * (k + 1)],
                                         start=(k == 0), stop=False)
                    for k in range(8):
                        nc.tensor.matmul(ps[:], hT[:, 64 * k:64 * (k + 1)],
                                         wh[:, 512 * k:512 * (k + 1)],
                                         start=False, stop=False)
                    nc.tensor.matmul(ps[:], ones1_sb[:],
                                     bt[:, 512 * n:512 * (n + 1)],
                                     start=False, stop=True)
                    func = AF.Tanh if n in (4, 5) else AF.Sigmoid
                    nc.scalar.activation(gates[:, 512 * n:512 * (n + 1)],
                                         ps[:], func)
                # c2 = sig_f*c + sig_i*tanh_g ; h2 = sig_o*tanh(c2)
                tmp = mid.tile([BC, H], F32, tag="lstm_tmp", name="tmp")
                nc.vector.tensor_tensor(tmp[:], gates[:, 0:H],
                                        gates[:, 2 * H:3 * H], ALU.mult)
                c2 = mid.tile([BC, H], F32, tag="c2t", name="c2")
                nc.vector.tensor_tensor(c2[:], gates[:, H:2 * H], c_in[:],
                                        ALU.mult)
                nc.vector.tensor_tensor(c2[:], c2[:], tmp[:], ALU.add)
                nc.sync.dma_start(d_cout[:], c2[:])
                tc2 = mid.tile([BC, H], F32, tag="lstm_tmp", name="tc2")
                nc.scalar.activation(tc2[:], c2[:], AF.Tanh)
                h2 = mid.tile([BC, H], F32, tag="h2t", name="h2")
                nc.vector.tensor_tensor(h2[:], gates[:, 3 * H:4 * H], tc2[:],
                                        ALU.mult)
                nc.sync.dma_start(d_hout[:], h2[:])
                h2f = mid.tile([BC, H], F16, tag="lstm_h2f", name="h2f")
                nc.vector.tensor_copy(h2f[:], h2[:])
                for j in range(8):
                    pt = psT.tile([128, BC], F16, tag="pT", name="trps")
                    nc.tensor.transpose(pt[:], h2f[:, 128 * j:128 * (j + 1)],
                                        I128_sb[0:64, 0:64])
                    nc.vector.tensor_copy(hTout[:, BC * j:BC * (j + 1)], pt[:])

            h0T_sb = mid.tile([128, 8 * BC], F16, tag="h0T")
            lstm_layer(xT_sb, 10, d_wi0, d_wh0, hT0_sb, d_b0, c0_sb,
                       d_h0n, d_c0n, h0T_sb, "l0", pre_wx0, pre_wh0)
            h1T_sb = mid.tile([128, 8 * BC], F16, tag="h1T")
            lstm_layer(h0T_sb, 8, d_wi1, d_wh1, hT1_sb, d_b1, c1_sb,
                       d_h1n, d_c1n, h1T_sb, "l1", {}, {})

            # ---------- phase I: logits ----------
            pl = psS.tile([BC, V], F32, tag="ps64", name="lps")
            for k in range(8):
                nc.tensor.matmul(pl[:], h1T_sb[:, 64 * k:64 * (k + 1)],
                                 owT_sb[k][:], start=(k == 0), stop=False)
            nc.tensor.matmul(pl[:], ones1_sb[:], ob_sb[:],
                             start=False, stop=True)
            lo = mid.tile([BC, V], F32, tag="lo")
            nc.vector.tensor_copy(lo[:], pl[:])
            nc.sync.dma_start(d_logits[:], lo[:])

    nc.compile()
    return nc


def _prep_inputs(input_ids, hidden, cell, encoder_outputs, emb, U, W, Vw,
                 Wih0, Whh0, bih0, bhh0, Wih1, Whh1, bih1, bhh1,
                 out_w, out_b):
    f16 = np.float16
    # shared across cores
    U16 = np.ascontiguousarray(U.astype(f16))
    W16 = np.ascontiguousarray(W.astype(f16))
    VwR = np.ascontiguousarray(Vw.reshape(8, 128).T.astype(f16))  # [128,8]
    emb16 = np.ascontiguousarray(emb.astype(f16))
    oh64 = np.zeros((BC, TOK), f16)
    for b in range(BC):
        oh64[b, 64 * b:64 * (b + 1)] = 1.0
    I128 = np.eye(128, dtype=f16)
    ones1 = np.ones((1, BC), f16)
    Wih0T = np.ascontiguousarray(Wih0.T.astype(f16))
    Whh0T = np.ascontiguousarray(Whh0.T.astype(f16))
    Wih1T = np.ascontiguousarray(Wih1.T.astype(f16))
    Whh1T = np.ascontiguousarray(Whh1.T.astype(f16))
    b0 = np.ascontiguousarray((bih0 + bhh0)[None, :].astype(f16))
    b1 = np.ascontiguousarray((bih1 + bhh1)[None, :].astype(f16))
    owT = np.ascontiguousarray(out_w.T.astype(f16))
    ob = np.ascontiguousarray(out_b[None, :].astype(f16))

    def blocked_T(x):  # [BC,H] -> [128, 8*BC] (k-blocks of columns)
        t = np.ascontiguousarray(x.T)          # [H, BC]
        return np.ascontiguousarray(
            t.reshape(8, 128, BC).transpose(1, 0, 2).reshape(128, 8 * BC)
        ).astype(f16)

    ids = np.asarray(input_ids).reshape(B)
    in_maps = []
    for c in range(NCORES):
        bs = slice(BC * c, BC * (c + 1))
        enc_c = encoder_outputs[bs]                      # [BC, S, H]
        encT = np.ascontiguousarray(
            enc_c.reshape(TOK, H).T.astype(f16))         # [H, TOK] b-major
        ohT = np.zeros((V, BC), f16)
        ohT[ids[bs].astype(np.int64), np.arange(BC)] = 1.0
        in_maps.append({
            "encT": encT, "Umat": U16, "Wmat": W16,
            "VwR": VwR,
            "hT0": blocked_T(hidden[0][bs]),
            "hT1": blocked_T(hidden[1][bs]),
            "onehotT": ohT, "embW": emb16, "oh64": oh64,
            "I128": I128, "ones1": ones1,
            "Wih0T": Wih0T, "Whh0T": Whh0T, "Wih1T": Wih1T, "Whh1T": Whh1T,
            "bias0": b0, "bias1": b1,
            "cell0": np.ascontiguousarray(cell[0][bs], dtype=np.float32),
            "cell1": np.ascontiguousarray(cell[1][bs], dtype=np.float32),
            "outWT": owT, "outB": ob,
        })
    return in_maps


def kernel(input_ids, hidden, cell, encoder_outputs, emb, U, W, Vw,
           Wih0, Whh0, bih0, bhh0, Wih1, Whh1, bih1, bhh1,
           out_w, out_b, matrix=0, _trace=False):
    if _COMPILED[0] is None:
        _COMPILED[0] = _build()
    nc = _COMPILED[0]
    args = [np.asarray(a) for a in
            (input_ids, hidden, cell, encoder_outputs, emb, U, W, Vw,
             Wih0, Whh0, bih0, bhh0, Wih1, Whh1, bih1, bhh1, out_w, out_b)]
    in_maps = _prep_inputs(*args)
    res = run_bass_kernel_spmd(nc, in_maps, core_ids=list(range(NCORES)),
                               trace=_trace)
    outs = res.results
    logits = np.concatenate([outs[c]["logits"] for c in range(NCORES)], 0)
    h_new = np.stack([
        np.concatenate([outs[c]["h0n"] for c in range(NCORES)], 0),
        np.concatenate([outs[c]["h1n"] for c in range(NCORES)], 0)])
    c_new = np.stack([
        np.concatenate([outs[c]["c0n"] for c in range(NCORES)], 0),
        np.concatenate([outs[c]["c1n"] for c in range(NCORES)], 0)])
    out = logits[:, None, :].astype(np.float32)
    kernel._last_results = res
    if int(np.asarray(matrix)):
        raise NotImplementedError("matrix=1 path not needed (reference uses 0)")
    return (out, h_new.astype(np.float32), c_new.astype(np.float32))


# revision 27
# speedup vs baseline: 1.2086x; 1.0102x over previous
"""Trainium2 Bass kernel for one attention-LSTM decoder step.

dims: B=512, S=64, H=1024, E=256, V=128, L=2, sharded data-parallel over
batch across 8 NeuronCores (64 batches/core). All matmuls run in fp16 with
fp32 PSUM accumulation; elementwise/softmax math in fp32.
"""

import sys

if "/opt/trn_rl_repo" not in sys.path:
    sys.path.insert(0, "/opt/trn_rl_repo")

import numpy as np

import concourse.bacc as bacc
import concourse.mybir as mybir
import concourse.tile as tile
from concourse.bass_utils import run_bass_kernel_spmd

B, S, H, E, V = 512, 64, 1024, 256, 128
NCORES = 8
BC = B // NCORES          # 64 batches per core
TOK = BC * S              # 4096 tokens per core
F = E + H                 # 1280 LSTM input features
G4 = 4 * H                # 4096 gate rows
F16 = mybir.dt.float16
F32 = mybir.dt.float32
AF = mybir.ActivationFunctionType
ALU = mybir.AluOpType
AX = mybir.AxisListType

_COMPILED = [None]


def _build():
    nc = bacc.Bacc("TRN2", target_bir_lowering=False, debug=False,
                   num_devices=NCORES)

    # ---- DRAM I/O ----
    d_encT = nc.dram_tensor("encT", [H, TOK], F16, kind="ExternalInput")
    d_U = nc.dram_tensor("Umat", [H, H], F16, kind="ExternalInput")
    d_W = nc.dram_tensor("Wmat", [H, H], F16, kind="ExternalInput")
    d_Vw = nc.dram_tensor("VwR", [128, 8], F16, kind="ExternalInput")
    d_hT0 = nc.dram_tensor("hT0", [128, 8 * BC], F16, kind="ExternalInput")
    d_hT1 = nc.dram_tensor("hT1", [128, 8 * BC], F16, kind="ExternalInput")
    d_ohT = nc.dram_tensor("onehotT", [V, BC], F16, kind="ExternalInput")
    d_emb = nc.dram_tensor("embW", [V, E], F16, kind="ExternalInput")
    d_oh64 = nc.dram_tensor("oh64", [BC, TOK], F16, kind="ExternalInput")
    d_I128 = nc.dram_tensor("I128", [128, 128], F16, kind="ExternalInput")
    d_ones1 = nc.dram_tensor("ones1", [1, BC], F16, kind="ExternalInput")
    d_wi0 = nc.dram_tensor("Wih0T", [F, G4], F16, kind="ExternalInput")
    d_wh0 = nc.dram_tensor("Whh0T", [H, G4], F16, kind="ExternalInput")
    d_wi1 = nc.dram_tensor("Wih1T", [H, G4], F16, kind="ExternalInput")
    d_wh1 = nc.dram_tensor("Whh1T", [H, G4], F16, kind="ExternalInput")
    d_b0 = nc.dram_tensor("bias0", [1, G4], F16, kind="ExternalInput")
    d_b1 = nc.dram_tensor("bias1", [1, G4], F16, kind="ExternalInput")
    d_c0 = nc.dram_tensor("cell0", [BC, H], F32, kind="ExternalInput")
    d_c1 = nc.dram_tensor("cell1", [BC, H], F32, kind="ExternalInput")
    d_owT = nc.dram_tensor("outWT", [H, V], F16, kind="ExternalInput")
    d_ob = nc.dram_tensor("outB", [1, V], F16, kind="ExternalInput")

    d_logits = nc.dram_tensor("logits", [BC, V], F32, kind="ExternalOutput")
    d_h0n = nc.dram_tensor("h0n", [BC, H], F32, kind="ExternalOutput")
    d_h1n = nc.dram_tensor("h1n", [BC, H], F32, kind="ExternalOutput")
    d_c0n = nc.dram_tensor("c0n", [BC, H], F32, kind="ExternalOutput")
    d_c1n = nc.dram_tensor("c1n", [BC, H], F32, kind="ExternalOutput")

    with tile.TileContext(nc) as tc:
        with (
            tc.tile_pool(name="const", bufs=1) as cpool,
            tc.tile_pool(name="stream", bufs=3) as stp,
            tc.tile_pool(name="tanh", bufs=3) as thp,
            tc.tile_pool(name="wls", bufs=3) as wpool,
            tc.tile_pool(name="mid", bufs=1) as mid,
            tc.tile_pool(name="psA", bufs=3, space="PSUM") as psA,
            tc.tile_pool(name="psE", bufs=1, space="PSUM") as psE,
            tc.tile_pool(name="psS", bufs=2, space="PSUM") as psS,
            tc.tile_pool(name="psT", bufs=2, space="PSUM") as psT,
        ):
            # ---------- resident constants ----------
            vw_sb = cpool.tile([128, 8], F16, tag="vw")
            nc.sync.dma_start(vw_sb[:], d_Vw[:])
            hT0_sb = cpool.tile([128, 8 * BC], F16, tag="hT0")
            nc.sync.dma_start(hT0_sb[:], d_hT0[:])
            hT1_sb = cpool.tile([128, 8 * BC], F16, tag="hT1")
            nc.sync.dma_start(hT1_sb[:], d_hT1[:])
            ohT_sb = cpool.tile([V, BC], F16, tag="ohT")
            nc.sync.dma_start(ohT_sb[:], d_ohT[:])
            emb_sb = cpool.tile([V, E], F16, tag="emb")
            nc.sync.dma_start(emb_sb[:], d_emb[:])
            ones1_sb = cpool.tile([1, BC], F16, tag="ones1")
            nc.sync.dma_start(ones1_sb[:], d_ones1[:])

            # ---------- phase A: t2 = h_top @ W  -> [BC, H] f16 ----------
            t2_sb = mid.tile([BC, H], F16, tag="t2")
            for n2 in range(2):
                ps = psS.tile([BC, 512], F32, tag="ps64", name="t2ps")
                wsl = stp.tile([128, 4096], F16, tag="big", name="wsl")
                for hf in range(2):
                    nc.sync.dma_start(
                        wsl[:, 2048 * hf:2048 * (hf + 1)]
                            .rearrange("p (k c) -> p k c", c=512),
                        d_W.rearrange("(k p) h -> k p h", p=128)
                           [4 * hf:4 * (hf + 1), :,
                            512 * n2:512 * (n2 + 1)]
                           .transpose([1, 0, 2]))
                for k in range(8):
                    nc.tensor.matmul(
                        ps[:], hT1_sb[:, 64 * k:64 * (k + 1)],
                        wsl[:, 512 * k:512 * (k + 1)],
                        start=(k == 0), stop=(k == 7))
                nc.vector.tensor_copy(t2_sb[:, 512 * n2:512 * (n2 + 1)], ps[:])

            U_sb = [cpool.tile([128, H], F16, tag=f"U{k}", name=f"U{k}") for k in range(8)]
            for k in range(8):
                nc.scalar.dma_start(U_sb[k][:], d_U[128 * k:128 * (k + 1), :])
            # ---------- phase B: embedded^T -> xT blocks 0..1 ----------
            xT_sb = mid.tile([128, 10 * BC], F16, tag="xT")
            for et in range(2):
                ps = psT.tile([128, BC], F32, tag="pT", name="embps")
                nc.tensor.matmul(ps[:], emb_sb[:, 128 * et:128 * (et + 1)],
                                 ohT_sb[:], start=True, stop=True)
                nc.vector.tensor_copy(xT_sb[:, BC * et:BC * (et + 1)], ps[:])

            def load_et(n):
                t = stp.tile([128, 4096], F16, tag="big", name=f"et{n}")
                nc.sync.dma_start(
                    t[:].rearrange("p (k c) -> p k c", c=512),
                    d_encT.rearrange("(k p) t -> k p t", p=128)
                          [:, :, 512 * n:512 * (n + 1)]
                          .transpose([1, 0, 2]))
                return t

            def load_oh(n):
                t = stp.tile([BC, 512], F16, tag="oh", name=f"oh{n}")
                nc.sync.dma_start(t[:], d_oh64[:, 512 * n:512 * (n + 1)])
                return t

            pre_et = {n: load_et(n) for n in range(2)}
            pre_oh = {n: load_oh(n) for n in range(2)}
            def load_w(wd, n_k, n, eng):
                t = wpool.tile([128, n_k * 512], F16,
                               tag="wx" if n_k > 8 else "wh", name=f"w{n}")
                eng.dma_start(
                    t[:].rearrange("p (k c) -> p k c", c=512),
                    wd.rearrange("(k p) g -> k p g", p=128)
                      [:, :, 512 * n:512 * (n + 1)].transpose([1, 0, 2]))
                return t

            pre_wx0 = {n: load_w(d_wi0, 10, n, nc.scalar) for n in range(3)}
            pre_wh0 = {n: load_w(d_wh0, 8, n, nc.sync) for n in range(3)}

            # ---------- phase C: t1 + t2 -> tanh -> scores -> softmax -> ct ---
            # token tile n holds ALL 64 s-positions of batches 8n..8n+8, so
            # each tile's softmax + attention-context can be computed inline
            # against the already-resident encT tile (no second enc stream).
            ctT_sb = mid.tile([128, 512], F32, tag="ctT")  # [h-blk k][8n+b] cols
            for n in range(8):          # token tiles (512 tokens, 8 batches)
                et = pre_et.pop(n) if n in pre_et else load_et(n)
                oh64_t = pre_oh.pop(n) if n in pre_oh else load_oh(n)
                pe = psE.tile([1, 512], F32, tag="eps")
                ths = []
                for m in range(8):      # output-H tiles
                    pt = psA.tile([128, 512], F32, tag="t1ps")
                    for k in range(8):
                        nc.tensor.matmul(
                            pt[:], U_sb[k][:, 128 * m:128 * (m + 1)],
                            et[:, 512 * k:512 * (k + 1)],
                            start=(k == 0), stop=False)
                    # inject t2 broadcast over s:  lhsT=[64b,128h'] rhs=[64b,512tok]
                    nc.tensor.matmul(
                        pt[:], t2_sb[:, 128 * m:128 * (m + 1)],
                        oh64_t[:], start=False, stop=True)
                    th = thp.tile([128, 512], F16, tag="tanh", name=f"th{m}",
                                  bufs=9)
                    nc.scalar.activation(th[:], pt[:], AF.Tanh)
                    ths.append(th)
                # batched scores: Vw stays loaded across the 8 matmuls
                for m in range(8):
                    nc.tensor.matmul(pe[:], vw_sb[:, m:m + 1], ths[m][:],
                                     start=(m == 0), stop=(m == 7))
                # --- inline softmax over s for batches 8n..8n+8 ---
                er = mid.tile([1, 512], F32, tag="er", name="er", bufs=2)
                nc.vector.tensor_copy(er[:], pe[:])
                eb = mid.tile([8, S], F32, tag="eb", name="eb", bufs=2)
                nc.scalar.dma_start(
                    eb[:], er[0:1, :].rearrange("p (b s) -> p b s", b=8))
                mx = mid.tile([8, 1], F32, tag="mx", name="mx", bufs=2)
                nc.vector.tensor_reduce(mx[:], eb[:], axis=AX.X, op=ALU.max)
                negmx = mid.tile([8, 1], F32, tag="negmx", name="negmx", bufs=2)
                nc.vector.tensor_scalar_mul(negmx[:], mx[:], -1.0)
                pb = mid.tile([8, S], F32, tag="pb", name="pb", bufs=2)
                nc.scalar.activation(pb[:], eb[:], AF.Exp, bias=negmx[:])
                sm = mid.tile([8, 1], F32, tag="sm", name="sm", bufs=2)
                nc.vector.tensor_reduce(sm[:], pb[:], axis=AX.X, op=ALU.add)
                rinv = mid.tile([8, 1], F32, tag="rinv", name="rinv", bufs=2)
                nc.vector.reciprocal(rinv[:], sm[:])
                ab = mid.tile([8, S], F16, tag="ab", name="ab", bufs=2)
                nc.vector.tensor_scalar_mul(ab[:], pb[:], rinv[:])
                ar = mid.tile([1, 512], F16, tag="ar", name="ar", bufs=2)
                nc.scalar.dma_start(
                    ar[0:1, :].rearrange("p (b s) -> p b s", b=8), ab[:])
                af = mid.tile([128, 512], F16, tag="af", name="af", bufs=2)
                nc.gpsimd.partition_broadcast(af[:], ar[:])
                # --- ct^T columns for these 8 batches, per h-block k ---
                for k in range(8):
                    pr = thp.tile([128, 512], F16, tag="prod", name="pr", bufs=3)
                    nc.vector.tensor_tensor(
                        pr[:], et[:, 512 * k:512 * (k + 1)], af[:], ALU.mult)
                    nc.vector.tensor_reduce(
                        ctT_sb[:, 64 * k + 8 * n:64 * k + 8 * n + 8],
                        pr[:].rearrange("p (b s) -> p b s", b=8),
                        axis=AX.X, op=ALU.add)
                    if n == 7:
                        # block k of ct^T is now complete -> xT block 2+k
                        nc.vector.tensor_copy(
                            xT_sb[:, BC * (2 + k):BC * (3 + k)],
                            ctT_sb[:, 64 * k:64 * (k + 1)])

            I128_sb = cpool.tile([128, 128], F16, tag="I128")
            nc.sync.dma_start(I128_sb[:], d_I128[:])
            c0_sb = cpool.tile([BC, H], F32, tag="c0")
            nc.sync.dma_start(c0_sb[:], d_c0[:])
            c1_sb = cpool.tile([BC, H], F32, tag="c1")
            nc.sync.dma_start(c1_sb[:], d_c1[:])
            owT_sb = [cpool.tile([128, V], F16, tag=f"ow{k}", name=f"ow{k}") for k in range(8)]
            for k in range(8):
                nc.sync.dma_start(owT_sb[k][:], d_owT[128 * k:128 * (k + 1), :])
            ob_sb = cpool.tile([1, V], F16, tag="ob")
            nc.sync.dma_start(ob_sb[:], d_ob[:])

            # ---------- phases G/H: two LSTM layers ----------
            def lstm_layer(xT, n_xk, wxd, whd, hT, bias_d, c_in,
                           d_hout, d_cout, hTout, lname, pre_wx, pre_wh):
                """xT: SBUF [128, n_xk*64] input^T blocks; whd/wxd DRAM weights;
                hT: SBUF [128, 8*64] prev-h^T blocks; returns nothing."""
                gates = mid.tile([BC, G4], F16, tag="gates", name="gates")
                bt = mid.tile([1, G4], F16, tag="btile", name="bt")
                nc.sync.dma_start(bt[:], bias_d[:])
                for n in range(8):
                    wx = pre_wx.pop(n) if n in pre_wx else \
                        load_w(wxd, n_xk, n, nc.scalar)
                    wh = pre_wh.pop(n) if n in pre_wh else \
                        load_w(whd, 8, n, nc.sync)
                    ps = psS.tile([BC, 512], F32, tag="ps64", name="gps")
                    for k in range(n_xk):
                        nc.tensor.matmul(ps[:], xT[:, 64 * k:64 * (k + 1)],
                                         wx[:, 512 * k:512 * (k + 1)],
                                         start=(k == 0), stop=False)
                    for k in range(8):
                        nc.tensor.matmul(ps[:], hT[:, 64 * k:64 * (k + 1)],
                                         wh[:, 512 * k:512 * (k + 1)],
                                         start=False, stop=False)
                    nc.tensor.matmul(ps[:], ones1_sb[:],
                                     bt[:, 512 * n:512 * (n + 1)],
                                     start=False, stop=True)
                    func = AF.Tanh if n in (4, 5) else AF.Sigmoid
                    nc.scalar.activation(gates[:, 512 * n:512 * (n + 1)],
                                         ps[:], func)
                # c2 = sig_f*c + sig_i*tanh_g ; h2 = sig_o*tanh(c2)
                tmp = mid.tile([BC, H], F32, tag="lstm_tmp", name="tmp")
                nc.vector.tensor_tensor(tmp[:], gates[:, 0:H],
                                        gates[:, 2 * H:3 * H], ALU.mult)
                c2 = mid.tile([BC, H], F32, tag="c2t", name="c2")
                nc.vector.tensor_tensor(c2[:], gates[:, H:2 * H], c_in[:],
                                        ALU.mult)
                nc.vector.tensor_tensor(c2[:], c2[:], tmp[:], ALU.add)
                nc.sync.dma_start(d_cout[:], c2[:])
                tc2 = mid.tile([BC, H], F32, tag="lstm_tmp", name="tc2")
                nc.scalar.activation(tc2[:], c2[:], AF.Tanh)
                h2 = mid.tile([BC, H], F32, tag="h2t", name="h2")
                nc.vector.tensor_tensor(h2[:], gates[:, 3 * H:4 * H], tc2[:],
                                        ALU.mult)
                nc.sync.dma_start(d_hout[:], h2[:])
                h2f = mid.tile([BC, H], F16, tag="lstm_h2f", name="h2f")
                nc.vector.tensor_copy(h2f[:], h2[:])
                for j in range(8):
                    pt = psT.tile([128, BC], F16, tag="pT", name="trps")
                    nc.tensor.transpose(pt[:], h2f[:, 128 * j:128 * (j + 1)],
                                        I128_sb[0:64, 0:64])
                    nc.vector.tensor_copy(hTout[:, BC * j:BC * (j + 1)], pt[:])

            h0T_sb = mid.tile([128, 8 * BC], F16, tag="h0T")
            lstm_layer(xT_sb, 10, d_wi0, d_wh0, hT0_sb, d_b0, c0_sb,
                       d_h0n, d_c0n, h0T_sb, "l0", pre_wx0, pre_wh0)
            h1T_sb = mid.tile([128, 8 * BC], F16, tag="h1T")
            lstm_layer(h0T_sb, 8, d_wi1, d_wh1, hT1_sb, d_b1, c1_sb,
                       d_h1n, d_c1n, h1T_sb, "l1", {}, {})

            # ---------- phase I: logits ----------
            pl = psS.tile([BC, V], F32, tag="ps64", name="lps")
            for k in range(8):
                nc.tensor.matmul(pl[:], h1T_sb[:, 64 * k:64 * (k + 1)],
                                 owT_sb[k][:], start=(k == 0), stop=False)
            nc.tensor.matmul(pl[:], ones1_sb[:], ob_sb[:],
                             start=False, stop=True)
            lo = mid.tile([BC, V], F32, tag="lo")
            nc.vector.tensor_copy(lo[:], pl[:])
            nc.sync.dma_start(d_logits[:], lo[:])

    nc.compile()
    return nc


def _prep_inputs(input_ids, hidden, cell, encoder_outputs, emb, U, W, Vw,
                 Wih0, Whh0, bih0, bhh0, Wih1, Whh1, bih1, bhh1,
                 out_w, out_b):
    f16 = np.float16
    # shared across cores
    U16 = np.ascontiguousarray(U.astype(f16))
    W16 = np.ascontiguousarray(W.astype(f16))
    VwR = np.ascontiguousarray(Vw.reshape(8, 128).T.astype(f16))  # [128,8]
    emb16 = np.ascontiguousarray(emb.astype(f16))
    oh64 = np.zeros((BC, TOK), f16)
    for b in range(BC):
        oh64[b, 64 * b:64 * (b + 1)] = 1.0
    I128 = np.eye(128, dtype=f16)
    ones1 = np.ones((1, BC), f16)
    Wih0T = np.ascontiguousarray(Wih0.T.astype(f16))
    Whh0T = np.ascontiguousarray(Whh0.T.astype(f16))
    Wih1T = np.ascontiguousarray(Wih1.T.astype(f16))
    Whh1T = np.ascontiguousarray(Whh1.T.astype(f16))
    b0 = np.ascontiguousarray((bih0 + bhh0)[None, :].astype(f16))
    b1 = np.ascontiguousarray((bih1 + bhh1)[None, :].astype(f16))
    owT = np.ascontiguousarray(out_w.T.astype(f16))
    ob = np.ascontiguousarray(out_b[None, :].astype(f16))

    def blocked_T(x):  # [BC,H] -> [128, 8*BC] (k-blocks of columns)
        t = np.ascontiguousarray(x.T)          # [H, BC]
        return np.ascontiguousarray(
            t.reshape(8, 128, BC).transpose(1, 0, 2).reshape(128, 8 * BC)
        ).astype(f16)

    ids = np.asarray(input_ids).reshape(B)
    in_maps = []
    for c in range(NCORES):
        bs = slice(BC * c, BC * (c + 1))
        enc_c = encoder_outputs[bs]                      # [BC, S, H]
        encT = np.ascontiguousarray(
            enc_c.reshape(TOK, H).T.astype(f16))         # [H, TOK] b-major
        ohT = np.zeros((V, BC), f16)
        ohT[ids[bs].astype(np.int64), np.arange(BC)] = 1.0
        in_maps.append({
            "encT": encT, "Umat": U16, "Wmat": W16,
            "VwR": VwR,
            "hT0": blocked_T(hidden[0][bs]),
            "hT1": blocked_T(hidden[1][bs]),
            "onehotT": ohT, "embW": emb16, "oh64": oh64,
            "I128": I128, "ones1": ones1,
            "Wih0T": Wih0T, "Whh0T": Whh0T, "Wih1T": Wih1T, "Whh1T": Whh1T,
            "bias0": b0, "bias1": b1,
            "cell0": np.ascontiguousarray(cell[0][bs], dtype=np.float32),
            "cell1": np.ascontiguousarray(cell[1][bs], dtype=np.float32),
            "outWT": owT, "outB": ob,
        })
    return in_maps


def kernel(input_ids, hidden, cell, encoder_outputs, emb, U, W, Vw,
           Wih0, Whh0, bih0, bhh0, Wih1, Whh1, bih1, bhh1,
           out_w, out_b, matrix=0, _trace=False):
    if _COMPILED[0] is None:
        _COMPILED[0] = _build()
    nc = _COMPILED[0]
    args = [np.asarray(a) for a in
            (input_ids, hidden, cell, encoder_outputs, emb, U, W, Vw,
             Wih0, Whh0, bih0, bhh0, Wih1, Whh1, bih1, bhh1, out_w, out_b)]
    in_maps = _prep_inputs(*args)
    res = run_bass_kernel_spmd(nc, in_maps, core_ids=list(range(NCORES)),
                               trace=_trace)
    outs = res.results
    logits = np.concatenate([outs[c]["logits"] for c in range(NCORES)], 0)
    h_new = np.stack([
        np.concatenate([outs[c]["h0n"] for c in range(NCORES)], 0),
        np.concatenate([outs[c]["h1n"] for c in range(NCORES)], 0)])
    c_new = np.stack([
        np.concatenate([outs[c]["c0n"] for c in range(NCORES)], 0),
        np.concatenate([outs[c]["c1n"] for c in range(NCORES)], 0)])
    out = logits[:, None, :].astype(np.float32)
    kernel._last_results = res
    if int(np.asarray(matrix)):
        raise NotImplementedError("matrix=1 path not needed (reference uses 0)")
    return (out, h_new.astype(np.float32), c_new.astype(np.float32))
